# revision 1
# baseline (speedup 1.0000x reference)
"""Trainium2 Bass kernel for nn_Attention (Gaussian banded attention).

Math (reference):
    v = values @ input_weights.T                      # [B,L,D]
    probs[h,q,k] = N(k - q - off_h; std_h)            # Gaussian, depends on k-q only
    attended[b,h,q,:] = sum_k probs[h,q,k] v[b,k,h*pd:(h+1)*pd]
    out = attended_merged @ output_weight.T           # [B,L,D]

Key structural facts exploited:
  - probs is a banded Toeplitz matrix per head: nonzero only for
    k - q in [off - 6*std, off + 6*std] (6-sigma truncation, error ~1e-8).
    Widest band: std=8, off=-8 -> k-q in [-56, 40].
  - So attention is a narrow depthwise convolution along L; no [L,L] matmul.
  - Batch x L sharding is embarrassingly parallel given a halo of
    56 backward / 40 forward rows of the INPUT (v is a row-wise projection,
    zero rows project to zero since there is no bias).

Sharding: 8 cores = (B=2) x (4 chunks of 512 rows of L). Each core gets
x.T zero-padded to [1024, 640] (56 halo + 512 + 40 halo + 32 zero pad),
computes in [D, L]->[L, D]->[D, L] layouts on the TensorEngine in bf16,
and writes out.T [1024, 512] bf16 (host casts back to f32 on reassembly).
No collectives.

Cost-model performance (CoreSim, TRN2 timing): 38,609 ns single execution;
33,046 ns/iter steady state = TensorE 100% busy (gapless streaming floor).
"""

import math
from contextlib import ExitStack

import numpy as np
import ml_dtypes

import concourse.bass as bass
from concourse import mybir
from concourse.bass_utils import run_bass_kernel_spmd

# ---- NEFF disk cache (keyed by BIR hash) to avoid recompiling identical
# graphs in fresh processes ----
import hashlib
import os
import shutil

_NEFF_CACHE_DIR = os.environ.get("NEFF_CACHE_DIR", "/root/neff_cache")


def _install_neff_cache():
    import concourse.bass_utils as _bu
    import concourse.bass2jax as _b2j
    if getattr(_bu, "_neff_cache_installed", False):
        return
    orig = _bu.compile_bir_kernel

    def cached(bir_json, tmpdir, neff_name="file.neff"):
        cpath = None
        try:
            os.makedirs(_NEFF_CACHE_DIR, exist_ok=True)
            key = hashlib.sha256(bir_json).hexdigest()[:32]
            cpath = os.path.join(_NEFF_CACHE_DIR, f"{key}.neff")
            dst = os.path.join(tmpdir, neff_name)
            if os.path.exists(cpath):
                shutil.copy(cpath, dst)
                return dst
        except OSError:
            cpath = None  # cache unusable; plain compile below
        path = orig(bir_json, tmpdir, neff_name)
        if cpath is not None:
            try:
                shutil.copy(path, cpath)
            except OSError:
                pass
        return path

    _bu.compile_bir_kernel = cached
    _b2j.compile_bir_kernel = cached
    _bu._neff_cache_installed = True


_install_neff_cache()

# ---------------- problem constants (hardcoded per spec) ----------------
B, L, D = 2, 2048, 1024
H, PD = 8, 128
ATTN_STD = np.array([1.0, 2.0, 4.0, 8.0, 1.0, 2.0, 4.0, 8.0], dtype=np.float64)
ATTN_OFFSET = np.array([-1.0, -2.0, -4.0, -8.0, -1.0, -2.0, -4.0, -8.0], dtype=np.float64)

N_CORES = 8
CHUNK = 512            # output rows per core
HALO_L, HALO_R = 56, 40
LPAD = 640             # 56 + 512 + 40 = 608, padded to 5*128
LT = 5                 # l-tiles of v (640 / 128)
KT = 8                 # d tiles (1024 / 128)
NQ = CHUNK             # query columns per core

BF16 = mybir.dt.bfloat16
F32 = mybir.dt.float32

G1 = LT * 2            # proj1 groups: (l-tile, n-chunk) -> v
G2 = H                 # attention heads -> attendedT
G3 = KT                # proj2 d_out tiles -> outT
NPS = 4                # rotating PSUM banks


def gauss_toeplitz_table() -> np.ndarray:
    """tp[h, r, m] = g_h(r - (m - 512) - 56), shape [H, 128, 1024] bf16.

    For v-tile t (rows k' = 128t + r of padded-local v) the attention rhs is
    tp[h][:, 512-128t : 1024-128t] so that rhs[r, q'] = g_h(128t + r - q' - 56),
    which is probs[h, q, k].T in padded-local coordinates.
    """
    r = np.arange(128, dtype=np.float64)[:, None]
    m = np.arange(1024, dtype=np.float64)[None, :]
    delta = r - (m - 512.0) - 56.0  # = k - q
    tables = []
    for h in range(H):
        std, off = ATTN_STD[h], ATTN_OFFSET[h]
        z = (delta - off) / std
        g = np.exp(-0.5 * z * z) / (std * math.sqrt(2.0 * math.pi))
        g[np.abs(z) > 6.0] = 0.0
        tables.append(g)
    return np.stack(tables).astype(ml_dtypes.bfloat16)


def attn_windows(h: int):
    """Static (t, j0, j1) list: nonzero q-column window of v-tile t for head h,
    8-aligned. Coverage of [0,512) is guaranteed (window width > 128)."""
    std, off = int(ATTN_STD[h]), int(ATTN_OFFSET[h])
    wlo = -56 - off - 6 * std
    whi = 71 - off + 6 * std
    res = []
    for t in range(LT):
        j0 = max(0, 128 * t + wlo)
        j1 = min(NQ, 128 * t + whi + 1)
        if j0 >= j1:
            continue
        j0 = (j0 // 8) * 8
        j1 = min(NQ, ((j1 + 7) // 8) * 8)
        res.append((t, j0, j1))
    return res


def build_graph(iters: int = 1, banded: bool = True) -> bass.Bass:
    """One SPMD core program. iters>1 repeats the whole kernel (including
    DMAs) with monotonically increasing semaphore thresholds, for timing.

    Phase structure per iteration (PE program order):
      warmup: 3x N=256 + 1x N=184 discarded matmuls on a zeroed tile during
              the first DMA's latency window (p-state ramp off the critical
              path; the last MM is sized to land just past data-readiness);
      wave A: v[:, 0:512]  = x @ W1a  -- k-outer over psum banks 0-4 so the
              PE streams while the xt/w1a DMAs arrive;
      wave B: v[:, 512:1024] = x @ W1b -- k-inner, data resident, banks [5,6,7,0,1];
      ph2:    attendedT per head, banded Toeplitz windows, banks [2,3,4,5];
      ph3:    outT = W2 @ attendedT, banks [0,1,6,7] (so the last output
              copies gate nothing until wave B of the NEXT iteration).
    Copies: wave A -> vector, wave B -> scalar, ph2/ph3 alternate engines;
    xt/w1 double-buffered so iterations pipeline with zero PE gaps.
    """
    nc = bass.Bass()

    xt = nc.declare_dram_parameter("xt", [D, LPAD], BF16, isOutput=False)
    w1t = nc.declare_dram_parameter("w1t", [D, D], BF16, isOutput=False)
    w2t = nc.declare_dram_parameter("w2t", [D, D], BF16, isOutput=False)
    tp = nc.declare_dram_parameter("tp", [H, 128, 1024], BF16, isOutput=False)
    out = nc.declare_dram_parameter("out", [D, NQ], BF16, isOutput=True)

    xt_r = xt[:].rearrange("(o p) f -> p o f", p=128)    # [128, 8, 640]
    w1_r = w1t[:].rearrange("(o p) f -> p o f", p=128)   # [128, 8, 1024]
    w2_r = w2t[:].rearrange("(o p) f -> p o f", p=128)   # [128, 8, 1024]
    tp_r = tp[:].rearrange("h p f -> p h f")             # [128, 8, 1024]

    with ExitStack() as ctx:
        e = ctx.enter_context
        xt_sb = e(nc.sbuf_tensor("xt_sb", [128, 2, KT, LPAD], BF16))
        w1_sb = e(nc.sbuf_tensor("w1_sb", [128, 2, KT, D], BF16))
        w2_sb = e(nc.sbuf_tensor("w2_sb", [128, KT, D], BF16))
        TP0, TPW = (408, 240) if banded else (0, 1024)
        tp_sb = e(nc.sbuf_tensor("tp_sb", [128, H, TPW], BF16))
        tp_src = tp_r[:, :, TP0:TP0 + TPW]
        v_sb = e(nc.sbuf_tensor("v_sb", [128, LT, D], BF16))
        at_sb = e(nc.sbuf_tensor("at_sb", [128, H, NQ], BF16))
        o_sb = e(nc.sbuf_tensor("o_sb", [128, KT, NQ], BF16))
        zdum = e(nc.sbuf_tensor("zdum", [128, 384], BF16))
        ps = [e(nc.psum_tensor(f"ps{i}", [128, 512], F32)) for i in range(8)]

        sem_names = (["zd", "mmA", "mm1", "mm2", "mm3", "tp_d",
                      "cpA", "cpB", "cp2v", "cp2s", "cp3v", "cp3s"]
                     + [f"xt_d{k}b{p}" for k in range(KT) for p in (0, 1)]
                     + [f"{n}b{p}" for n in ("w1a_d0", "w1a_g1", "w1a_g2",
                                             "w1b_g1", "w1b_g2") for p in (0, 1)]
                     + ["w2_g1", "w2_g2"]
                     + [f"dmo{m}" for m in range(G3)])
        sems = {n: e(nc.semaphore(n)) for n in sem_names}

        WAVE_B_BANKS = [5, 6, 7, 0, 1]
        PH2_BANKS = [2, 3, 4, 5]
        PH3_BANKS = [0, 1, 6, 7]

        def cp2_sem(h):
            return sems["cp2v" if h % 2 == 0 else "cp2s"]

        def cp2_count(h, it):
            return it * 4 + h // 2 + 1

        def cp3_waits(m, it):
            """(sem, count) pairs proving ph3 group m is fully copied out."""
            s = sems["cp3v" if m % 2 == 0 else "cp3s"]
            return [(s, it * 4 + m // 2 + 1)]

        def cp3_sem(m):
            return cp3_waits(m, 0)[0][0]

        def cp3_count(m, it):
            return cp3_waits(m, it)[0][1]

        with nc.Block() as block:

            @block.sync
            def _(sync: bass.BassEngine):
                for it in range(iters):
                    buf = it % 2
                    if it > 1:
                        # xt/w1 buffer reuse: wave B (last reader) of iter it-2
                        sync.wait_ge(sems["mm1"], (it - 1) * LT)
                    def xt_dma(k):
                        sync.dma_start(out=xt_sb[:, buf, k, :],
                                       in_=xt_r[:, k, :]).then_inc(
                            sems[f"xt_d{k}b{buf}"], 16)

                    # schedule tuned so the HWDGE generator (625ns/DMA, shared)
                    # stays ahead of wave A's per-k consumption
                    xt_dma(0)
                    sync.dma_start(out=w1_sb[:, buf, 0, 0:512],
                                   in_=w1_r[:, 0, 0:512]).then_inc(
                        sems[f"w1a_d0b{buf}"], 16)
                    xt_dma(1)
                    sync.dma_start(out=w1_sb[:, buf, 1:4, 0:512],
                                   in_=w1_r[:, 1:4, 0:512]).then_inc(
                        sems[f"w1a_g1b{buf}"], 16)
                    xt_dma(2)
                    xt_dma(3)
                    sync.dma_start(out=w1_sb[:, buf, 4:8, 0:512],
                                   in_=w1_r[:, 4:8, 0:512]).then_inc(
                        sems[f"w1a_g2b{buf}"], 16)
                    for k in range(4, KT):
                        xt_dma(k)
                    sync.dma_start(out=w1_sb[:, buf, 0:4, 512:1024],
                                   in_=w1_r[:, 0:4, 512:1024]).then_inc(
                        sems[f"w1b_g1b{buf}"], 16)
                    sync.dma_start(out=w1_sb[:, buf, 4:8, 512:1024],
                                   in_=w1_r[:, 4:8, 512:1024]).then_inc(
                        sems[f"w1b_g2b{buf}"], 16)
                    if it == 0:
                        sync.dma_start(out=tp_sb[:], in_=tp_src).then_inc(
                            sems["tp_d"], 16)
                    if it > 0:
                        sync.wait_ge(sems["mm3"], it * G3)
                    sync.dma_start(out=w2_sb[:, 0:4, :],
                                   in_=w2_r[:, 0:4, :]).then_inc(sems["w2_g1"], 16)
                    sync.dma_start(out=w2_sb[:, 4:8, :],
                                   in_=w2_r[:, 4:8, :]).then_inc(sems["w2_g2"], 16)


            @block.tensor
            def _(tensor: bass.BassEngine):
                # HAM/p-state warmup: discarded matmuls into bank 0 while the
                # first input DMAs are in flight (wave A k=0 start=True clears)
                tensor.wait_ge(sems["zd"], 1)
                for _ in range(3):
                    tensor.matmul(ps[0][:, 0:256], zdum[:, 0:128],
                                  zdum[:, 128:384], start=True, stop=True)
                # final warmup trimmed to N=184 so the PE arrives at the wave-A
                # wait cluster just after data-readiness (the cost model defers
                # dispatch by ~1.6us if the PE arrives early - measured cliff)
                tensor.matmul(ps[0][:, 0:184], zdum[:, 0:128],
                              zdum[:, 128:312], start=True, stop=True)
                for it in range(iters):
                    buf = it % 2
                    # ---- wave A: v[:, 0:512], k-outer, banks 0-4 ----
                    # cross-iter bank WAR: last users in iter it-1 were
                    # ph2 (banks 2,3,4 via h=4,5,6; bank 5 via h=7) and
                    # ph3 (banks 6,7,0,1 via m=4,5,6,7)
                    nth = (it // 2 + 1) * 16  # per-parity DMA count
                    for k in range(KT):
                        tensor.wait_ge(sems[f"xt_d{k}b{buf}"], nth)
                        if k == 0:
                            tensor.wait_ge(sems[f"w1a_d0b{buf}"], nth)
                        elif k == 1:
                            tensor.wait_ge(sems[f"w1a_g1b{buf}"], nth)
                        elif k == 4:
                            tensor.wait_ge(sems[f"w1a_g2b{buf}"], nth)
                        for lt in range(LT):
                            if k == 0 and it > 0:
                                if lt == 0:
                                    for s, c in cp3_waits(4, it - 1):
                                        tensor.wait_ge(s, c)
                                elif lt == 1:
                                    for s, c in cp3_waits(5, it - 1):
                                        tensor.wait_ge(s, c)
                                else:  # banks 2,3,4 <- ph2 h=4,5,6
                                    tensor.wait_ge(cp2_sem(lt + 2),
                                                   cp2_count(lt + 2, it - 1))
                            mm = tensor.matmul(
                                ps[lt][:, :],
                                xt_sb[:, buf, k, 128 * lt:128 * lt + 128],
                                w1_sb[:, buf, k, 0:512],
                                start=(k == 0), stop=(k == KT - 1),
                            )
                            if k == KT - 1:
                                mm.then_inc(sems["mmA"])
                    # ---- wave B: v[:, 512:1024], k-inner, banks [5,6,7,0,1] ----
                    for lt in range(LT):
                        bank = ps[WAVE_B_BANKS[lt]]
                        if lt == 0:
                            if it > 0:  # bank 5 <- ph2 h=7 of prev iter
                                tensor.wait_ge(cp2_sem(7), cp2_count(7, it - 1))
                        elif lt == 1:
                            if it > 0:  # bank 6 <- ph3 m=6 of prev iter
                                for s, c in cp3_waits(6, it - 1):
                                    tensor.wait_ge(s, c)
                        elif lt == 2:
                            if it > 0:  # bank 7 <- ph3 m=7 of prev iter
                                for s, c in cp3_waits(7, it - 1):
                                    tensor.wait_ge(s, c)
                        elif lt == 3:
                            # bank 0 <- wave A lt=0 copy of this iter
                            tensor.wait_ge(sems["cpA"], it * LT + 1)
                        else:
                            # bank 1 <- wave A lt=1 copy of this iter
                            tensor.wait_ge(sems["cpA"], it * LT + 2)
                        for k in range(KT):
                            if lt == 0 and k == 0:
                                tensor.wait_ge(sems[f"w1b_g1b{buf}"], nth)
                            elif lt == 0 and k == 4:
                                tensor.wait_ge(sems[f"w1b_g2b{buf}"], nth)
                            mm = tensor.matmul(
                                bank[:, :],
                                xt_sb[:, buf, k, 128 * lt:128 * lt + 128],
                                w1_sb[:, buf, k, 512:1024],
                                start=(k == 0), stop=(k == KT - 1),
                            )
                            if k == KT - 1:
                                mm.then_inc(sems["mm1"])

                    # ---- phase 2: attendedT per head (banded), banks 0-3 ----
                    if it == 0:
                        tensor.wait_ge(sems["tp_d"], 16)
                    for h in range(G2):
                        bank = ps[PH2_BANKS[h % 4]]
                        # bank WAR: banks 2,3,4 <- wave A lt=2,3,4 copies;
                        # bank 5 <- wave B g=0 copy; h>=4 <- ph2 head h-4
                        if h == 0:
                            tensor.wait_ge(sems["cpA"], it * LT + 3)
                        elif h == 1:
                            tensor.wait_ge(sems["cpA"], it * LT + 4)
                        elif h == 2:
                            tensor.wait_ge(sems["cpA"], it * LT + 5)
                        elif h == 3:
                            tensor.wait_ge(sems["cpB"], it * LT + 1)
                        else:
                            tensor.wait_ge(cp2_sem(h - 4), cp2_count(h - 4, it))
                        windows = attn_windows(h) if banded else [
                            (t, 0, NQ) for t in range(LT)]
                        for wi, (t, j0, j1) in enumerate(windows):
                            # data: v tile t, n-chunk h//4
                            if h // 4 == 0:
                                tensor.wait_ge(sems["cpA"], it * LT + t + 1)
                            else:
                                tensor.wait_ge(sems["cpB"], it * LT + t + 1)
                            c0 = 512 - 128 * t + j0 - TP0
                            c1 = 512 - 128 * t + j1 - TP0
                            mm = tensor.matmul(
                                bank[:, j0:j1],
                                v_sb[:, t, 128 * h:128 * h + 128],
                                tp_sb[:, h, c0:c1],
                                start=(wi == 0), stop=(wi == len(windows) - 1),
                            )
                            if wi == len(windows) - 1:
                                mm.then_inc(sems["mm2"])

                    # ---- phase 3: outT = W2 @ attendedT, banks 0-3 ----
                    for m in range(G3):
                        bank = ps[PH3_BANKS[m % 4]]
                        # bank WAR: banks 6,7,0,1 <- wave B g=1,2,3,4 copies;
                        # m>=4 <- ph3 copy m-4
                        if m < 4:
                            # banks [0,1,6,7] freed by wave B groups [3,4,1,2]
                            tensor.wait_ge(sems["cpB"],
                                           it * LT + [4, 5, 2, 3][m])
                        else:
                            for s, c in cp3_waits(m - 4, it):
                                tensor.wait_ge(s, c)
                        for k in range(KT):
                            if m == 0:
                                tensor.wait_ge(cp2_sem(k), cp2_count(k, it))
                                if k == 0:
                                    tensor.wait_ge(sems["w2_g1"], (it + 1) * 16)
                                elif k == 4:
                                    tensor.wait_ge(sems["w2_g2"], (it + 1) * 16)
                            mm = tensor.matmul(
                                bank[:, :],
                                w2_sb[:, k, 128 * m:128 * m + 128],
                                at_sb[:, k, :],
                                start=(k == 0), stop=(k == KT - 1),
                            )
                            if k == KT - 1:
                                mm.then_inc(sems["mm3"])

            @block.vector
            def _(vector: bass.BassEngine):
                for it in range(iters):
                    # wave A copies: v[:, lt, 0:512], banks 0-4, after k=7 MM
                    for lt in range(LT):
                        vector.wait_ge(sems["mmA"], it * LT + lt + 1)
                        vector.tensor_copy(
                            out=v_sb[:, lt, 0:512], in_=ps[lt][:, :],
                        ).then_inc(sems["cpA"])
                    for h in range(G2):
                        if h % 2 != 0:
                            continue
                        vector.wait_ge(sems["mm2"], it * G2 + h + 1)
                        vector.tensor_copy(
                            out=at_sb[:, h, :], in_=ps[PH2_BANKS[h % 4]][:, :],
                        ).then_inc(sems["cp2v"])
                    for m in [0, 2, 4, 6]:
                        vector.wait_ge(sems["mm3"], it * G3 + m + 1)
                        if it > 0:
                            vector.wait_ge(sems[f"dmo{m}"], it * 16)
                        vector.tensor_copy(
                            out=o_sb[:, m, :],
                            in_=ps[PH3_BANKS[m % 4]][:, :],
                        ).then_inc(sems["cp3v"])

            @block.gpsimd
            def _(gpsimd: bass.BassEngine):
                # zero the PE-warmup tile before anything else
                gpsimd.memset(zdum[:], 0).then_inc(sems["zd"])
                for it in range(iters):
                    for m in range(G3):
                        for s, c in cp3_waits(m, it):
                            gpsimd.wait_ge(s, c)
                        gpsimd.dma_start(
                            out=out[128 * m:128 * m + 128, :],
                            in_=o_sb[:, m, :],
                        ).then_inc(sems[f"dmo{m}"], 16)
                for m in range(G3):
                    gpsimd.wait_ge(sems[f"dmo{m}"], iters * 16)

            @block.scalar
            def _(scalar: bass.BassEngine):
                for it in range(iters):
                    # wave B copies: v[:, lt, 512:1024] from banks [4,5,6,7,4]
                    for lt in range(LT):
                        scalar.wait_ge(sems["mm1"], it * LT + lt + 1)
                        scalar.copy(v_sb[:, lt, 512:1024],
                                    ps[WAVE_B_BANKS[lt]][:, :]).then_inc(sems["cpB"])
                    for h in range(G2):
                        if h % 2 != 1:
                            continue
                        scalar.wait_ge(sems["mm2"], it * G2 + h + 1)
                        scalar.copy(at_sb[:, h, :],
                                    ps[PH2_BANKS[h % 4]][:, :]).then_inc(sems["cp2s"])
                    for m in [1, 3, 5, 7]:
                        scalar.wait_ge(sems["mm3"], it * G3 + m + 1)
                        if it > 0:
                            scalar.wait_ge(sems[f"dmo{m}"], it * 16)
                        scalar.copy(o_sb[:, m, :],
                                    ps[PH3_BANKS[m % 4]][:, :]).then_inc(
                            sems["cp3s"])


    return nc


# ---------------- host side ----------------

_GRAPH_CACHE: dict = {}


def get_graph(iters: int = 1, banded: bool = True) -> bass.Bass:
    key = (iters, banded)
    if key not in _GRAPH_CACHE:
        _GRAPH_CACHE[key] = build_graph(iters, banded)
    return _GRAPH_CACHE[key]


class Runner:
    """Compile-once executor for one Bass graph across the 8 cores.

    Mirrors bass2jax.run_bass_via_pjrt but keeps the jitted callable so
    repeated invocations don't re-trace/re-compile.
    """

    def __init__(self, nc: bass.Bass, n_cores: int = N_CORES):
        import jax
        from jax.sharding import Mesh, PartitionSpec
        from jax.experimental.shard_map import shard_map
        from concourse import bass2jax, mybir as _mb

        bass2jax.install_neuronx_cc_hook()
        self.n_cores = n_cores

        partition_name = (nc.partition_id_tensor.name
                          if nc.partition_id_tensor else None)
        in_names, out_names, out_avals, zero_shapes = [], [], [], []
        for alloc in nc.m.functions[0].allocations:
            if not isinstance(alloc, _mb.MemoryLocationSet):
                continue
            name = alloc.memorylocations[0].name
            if alloc.kind == "ExternalInput":
                if name != partition_name:
                    in_names.append(name)
            elif alloc.kind == "ExternalOutput":
                out_names.append(name)
                shape = tuple(alloc.tensor_shape)
                dtype = _mb.dt.np(alloc.dtype)
                out_avals.append(jax.core.ShapedArray(shape, dtype))
                zero_shapes.append((shape, dtype))
        self.in_names = list(in_names)
        self.out_names = out_names
        self.out_avals = out_avals
        self.zero_shapes = zero_shapes
        n_params = len(in_names)
        all_names = in_names + out_names
        if partition_name is not None:
            all_names = all_names + [partition_name]

        def _body(*args):
            operands = list(args)
            if partition_name is not None:
                operands.append(bass2jax.partition_id_tensor())
            outs = bass2jax._bass_exec_p.bind(
                *operands,
                out_avals=tuple(out_avals),
                in_names=tuple(all_names),
                out_names=tuple(out_names),
                lowering_input_output_aliases=(),
                sim_require_finite=True,
                sim_require_nnan=True,
                nc=nc,
            )
            return tuple(outs)

        devices = jax.devices()[:n_cores]
        mesh = Mesh(np.asarray(devices), ("core",))
        self._mesh = mesh
        n_outs = len(out_names)
        self._fn = jax.jit(
            shard_map(_body, mesh=mesh,
                      in_specs=(PartitionSpec("core"),) * (n_params + n_outs),
                      out_specs=(PartitionSpec("core"),) * n_outs,
                      check_rep=False),
            donate_argnums=tuple(range(n_params, n_params + n_outs)),
            keep_unused=True,
        )

    def stage(self, in_maps):
        """device_put the concatenated inputs once; returns device arrays."""
        import jax
        concat_in = [
            np.concatenate([np.asarray(m[name]) for m in in_maps], axis=0)
            for name in self.in_names
        ]
        return [jax.device_put(a) for a in concat_in]

    def make_zeros(self):
        if not hasattr(self, "_zeros_fn"):
            import jax
            import jax.numpy as jnp
            from jax.sharding import NamedSharding, PartitionSpec
            shardings = tuple(
                NamedSharding(self._mesh, PartitionSpec("core"))
                for _ in self.zero_shapes)
            shapes = [((self.n_cores * s[0], *s[1:]), d)
                      for s, d in self.zero_shapes]

            def _mk():
                return tuple(jnp.zeros(sh, dt) for sh, dt in shapes)

            self._zeros_fn = jax.jit(_mk, out_shardings=shardings)
        return list(self._zeros_fn())

    def run_staged(self, dev_in, dev_zeros):
        return self._fn(*dev_in, *dev_zeros)

    def __call__(self, in_maps):
        out_arrs = self._fn(*self.stage(in_maps), *self.make_zeros())
        return [
            {name: np.asarray(out_arrs[i]).reshape(
                self.n_cores, *self.out_avals[i].shape)[c]
             for i, name in enumerate(self.out_names)}
            for c in range(self.n_cores)
        ]


_RUNNER_CACHE: dict = {}


def get_runner(iters: int = 1) -> "Runner":
    if iters not in _RUNNER_CACHE:
        _RUNNER_CACHE[iters] = Runner(get_graph(iters))
    return _RUNNER_CACHE[iters]


def make_in_maps(values: np.ndarray, input_weights: np.ndarray,
                 output_weight: np.ndarray) -> list:
    bf = ml_dtypes.bfloat16
    w1t = np.ascontiguousarray(input_weights.T).astype(bf)
    w2t = np.ascontiguousarray(output_weight.T).astype(bf)
    tpt = gauss_toeplitz_table()
    in_maps = []
    for core in range(N_CORES):
        b, c = divmod(core, 4)
        lo, hi = c * CHUNK - HALO_L, c * CHUNK + CHUNK + HALO_R
        src_lo, src_hi = max(lo, 0), min(hi, L)
        xt_pad = np.zeros((D, LPAD), dtype=bf)
        xt_pad[:, src_lo - lo:src_hi - lo] = values[b, src_lo:src_hi, :].T.astype(bf)
        in_maps.append({"xt": xt_pad, "w1t": w1t, "w2t": w2t, "tp": tpt})
    return in_maps


def assemble(results: list) -> np.ndarray:
    out = np.empty((B, L, D), dtype=np.float32)
    for core in range(N_CORES):
        b, c = divmod(core, 4)
        out[b, c * CHUNK:(c + 1) * CHUNK, :] = \
            results[core]["out"].T.astype(np.float32)
    return out


def kernel(values: np.ndarray, input_weights: np.ndarray,
           output_weight: np.ndarray) -> np.ndarray:
    in_maps = make_in_maps(values, input_weights, output_weight)
    try:
        return assemble(get_runner(1)(in_maps))
    except Exception:
        # fallback: canonical SPMD path (re-traces per call but always works)
        res = run_bass_kernel_spmd(get_graph(1), in_maps,
                                   core_ids=list(range(N_CORES)))
        return assemble(res.results)



# revision 120
# speedup vs baseline: 1.2318x; 1.2318x over previous
"""Trainium2 Bass kernel for nn_Attention (Gaussian banded attention).

Math (reference):
    v = values @ input_weights.T                      # [B,L,D]
    probs[h,q,k] = N(k - q - off_h; std_h)            # Gaussian, depends on k-q only
    attended[b,h,q,:] = sum_k probs[h,q,k] v[b,k,h*pd:(h+1)*pd]
    out = attended_merged @ output_weight.T           # [B,L,D]

Key structural facts exploited:
  - probs is a banded Toeplitz matrix per head: nonzero only for
    k - q in [off - 6*std, off + 6*std] (6-sigma truncation, error ~1e-8).
    Widest band: std=8, off=-8 -> k-q in [-56, 40].
  - So attention is a narrow depthwise convolution along L; no [L,L] matmul.
  - Batch x L sharding is embarrassingly parallel given a halo of
    56 backward / 40 forward rows of the INPUT (v is a row-wise projection,
    zero rows project to zero since there is no bias).

Precision/speed scheme: both dense projections run on the PE in fp8e4
(e4m3) with perf_mode=DoubleRow (K=256 per instruction at 0.5 cycles per
output column -> 4x the bf16 FLOP rate), using a 3-term error-compensated
expansion per projection:
    X @ W  ~=  X_hi@W_hi + X_lo@W_hi + X_hi@W_lo
with X = X_hi + X_lo (hi = fp8(X), lo = fp8(X - X_hi)) and W shipped
pre-scaled by 64 (w values ~0.02 would be fp8-subnormal unscaled). The
64x output scale is absorbed into the Gaussian table (tp/64) after
attention, and divided out on the host for the final output. Attention
itself stays bf16 (small cost). Measured end-to-end rel err ~4e-3, same
as the all-bf16 variant.

Sharding: 8 cores = (B=2) x (4 chunks of 512 rows of L). Each core gets
x.T as stacked fp8 hi/lo [2048, 640] (56 halo + 512 + 40 halo + 32 zero
pad), computes in [D, L]->[L, D]->[D, L] layouts, and writes out.T
[1024, 512] bf16 carrying a 64x scale (host casts/divides on
reassembly). No collectives.

Schedule highlights (see build_graph):
  - attended readout: per head, one engine (scalar evens / vector odds)
    copies psum -> at_hi (fp8) then psum -> at_bf (bf16); the pool derives
    at_lo = fp8(at_bf - at_hi) in SBUF (the pool cannot access psum).
  - ph3 interleaves with ph2: the at_hi-only terms (T1/T3) for output
    tiles m0-3 run between ph2 head groups, so no psum bank or readout is
    ever awaited for long; groups m0-7 close staggered in a final m-outer
    k2=2,3 drain that overlaps the copy/DMA pipeline.
  - DMA generation is spread over three queues: sync (xt + next-iter w1),
    pool/SWDGE (iter-0 weights, w2, outputs m0-6), scalar (iter-0 xt lo
    half + the tail-critical m7 output, dodging the SWDGE end drain).
  - warmup matmuls keep the PE busy from t~0 (the cost model's p-state
    ramp is wall-clock based and a cold wait defers dispatch ~1.6us).

Cost-model performance: 31,343 ns single-shot; 26,134 ns/iter steady
state (PE ~99% busy; bf16 floor would be ~33,000 ns).
"""

import math
from contextlib import ExitStack

import numpy as np
import ml_dtypes

import concourse.bass as bass
from concourse import mybir
from concourse.bass_utils import run_bass_kernel_spmd

# ---- NEFF disk cache (keyed by BIR hash) to avoid recompiling identical
# graphs in fresh processes ----
import hashlib
import os
import shutil

_NEFF_CACHE_DIR = os.environ.get("NEFF_CACHE_DIR", "/root/neff_cache")


def _install_neff_cache():
    import concourse.bass_utils as _bu
    import concourse.bass2jax as _b2j
    if getattr(_bu, "_neff_cache_installed", False):
        return
    orig = _bu.compile_bir_kernel

    def cached(bir_json, tmpdir, neff_name="file.neff"):
        cpath = None
        try:
            os.makedirs(_NEFF_CACHE_DIR, exist_ok=True)
            key = hashlib.sha256(bir_json).hexdigest()[:32]
            cpath = os.path.join(_NEFF_CACHE_DIR, f"{key}.neff")
            dst = os.path.join(tmpdir, neff_name)
            if os.path.exists(cpath):
                shutil.copy(cpath, dst)
                return dst
        except OSError:
            cpath = None  # cache unusable; plain compile below
        path = orig(bir_json, tmpdir, neff_name)
        if cpath is not None:
            try:
                shutil.copy(path, cpath)
            except OSError:
                pass
        return path

    _bu.compile_bir_kernel = cached
    _b2j.compile_bir_kernel = cached
    _bu._neff_cache_installed = True


_install_neff_cache()

# ---------------- problem constants (hardcoded per spec) ----------------
B, L, D = 2, 2048, 1024
H, PD = 8, 128
ATTN_STD = np.array([1.0, 2.0, 4.0, 8.0, 1.0, 2.0, 4.0, 8.0], dtype=np.float64)
ATTN_OFFSET = np.array([-1.0, -2.0, -4.0, -8.0, -1.0, -2.0, -4.0, -8.0], dtype=np.float64)

N_CORES = 8
CHUNK = 512            # output rows per core
HALO_L, HALO_R = 56, 40
LPAD = 640             # 56 + 512 + 40 = 608, padded to 5*128
LT = 5                 # l-tiles of v (640 / 128)
KT = 8                 # d tiles (1024 / 128)
K2T = 4                # DoubleRow k-pair tiles (1024 / 256)
NQ = CHUNK             # query columns per core

BF16 = mybir.dt.bfloat16
F32 = mybir.dt.float32
FP8 = mybir.dt.float8e4
NP_FP8 = ml_dtypes.float8_e4m3
WSCALE = 64.0          # fp8 weights are shipped as 64*W

G2 = H                 # attention heads -> attendedT
G3 = KT                # proj2 d_out tiles -> outT
DR = mybir.MatmulPerfMode.DoubleRow

# (x-split, w-split) term order for the 3-term compensated product:
# (hi,hi), (lo,hi), (hi,lo) -- s index 0=hi, 1=lo
TERMS = ((0, 0), (1, 0), (0, 1))

# warmup matmul N sizes (mid-pstate: ~0.833 ns per column), tuned so the
# last one ends just past first-data-readiness (~920ns)
WARMUP_NS = (256, 224)


def gauss_toeplitz_table() -> np.ndarray:
    """tp[h, r, m] = g_h(r - (m - 512) - 56) / 64, shape [H, 128, 1024] bf16.

    For v-tile t (rows k' = 128t + r of padded-local v) the attention rhs is
    tp[h][:, 512-128t : 1024-128t] so that rhs[r, q'] = g_h(128t + r - q' - 56),
    which is probs[h, q, k].T in padded-local coordinates. The /64 cancels
    the 64x scale carried by v (fp8 weights are shipped as 64*W1^T).
    """
    r = np.arange(128, dtype=np.float64)[:, None]
    m = np.arange(1024, dtype=np.float64)[None, :]
    delta = r - (m - 512.0) - 56.0  # = k - q
    tables = []
    for h in range(H):
        std, off = ATTN_STD[h], ATTN_OFFSET[h]
        z = (delta - off) / std
        g = np.exp(-0.5 * z * z) / (std * math.sqrt(2.0 * math.pi))
        g[np.abs(z) > 6.0] = 0.0
        tables.append(g / WSCALE)
    return np.stack(tables).astype(ml_dtypes.bfloat16)


def attn_windows(h: int):
    """Static (t, j0, j1) list: nonzero q-column window of v-tile t for head h,
    8-aligned. Coverage of [0,512) is guaranteed (window width > 128).
    5-sigma truncation (leaked mass ~3e-6; the tp table is zeroed at 6)."""
    std, off = int(ATTN_STD[h]), int(ATTN_OFFSET[h])
    wlo = -56 - off - 5 * std
    whi = 71 - off + 5 * std
    res = []
    for t in range(LT):
        j0 = max(0, 128 * t + wlo)
        j1 = min(NQ, 128 * t + whi + 1)
        if j0 >= j1:
            continue
        j0 = (j0 // 8) * 8
        j1 = min(NQ, ((j1 + 7) // 8) * 8)
        res.append((t, j0, j1))
    return res


def build_graph(iters: int = 1, banded: bool = True) -> bass.Bass:
    """One SPMD core program. iters>1 repeats the whole kernel (including
    DMAs) with monotonically increasing semaphore thresholds, for timing.

    Phase structure per iteration (PE program order):
      warmup: discarded matmuls on a zeroed tile during the first DMA's
              latency window (p-state ramp off the critical path);
      wave A: v[:, 0:512]  = x @ W1a  -- k-pair-outer over psum banks 0-4 so
              the PE streams while the xt/w1a DMAs arrive; 3 fp8 DoubleRow
              terms per (k-pair, l-tile);
      wave B: v[:, 512:1024] = x @ W1b -- k-inner, data resident, banks [5,6,7,0,1];
      ph2/ph3 interleaved:
        ph2 h0-3 (banks 2-5, bf16 banded windows) -> ph3 T1/T3 for m0-3 at
        k2=0,1 (banks 0,1,6,7; needs only at_hi of h0-3) -> ph2 h4-7 ->
        deferred ph3 T2 terms for m0-3 -> ph3 m4-7 (k2=0,1; banks 2-5) ->
        drain: m-outer k2=2,3, all 8 groups close staggered.
    Readout per head: at_hi (fp8) + at_bf (bf16) psum copies on one engine
    (scalar: even heads, vector: odd), at_lo = fp8(at_bf - at_hi) on the
    pool in SBUF. Wave copies: A -> vector, B lt0-3 -> scalar, B lt4 ->
    vector; ph3 o_sb copies alternate engines; xt/w1 double-buffered so
    iterations pipeline.
    """
    nc = bass.Bass()

    xt = nc.declare_dram_parameter("xt", [2 * D, LPAD], FP8, isOutput=False)
    w1t = nc.declare_dram_parameter("w1t", [2 * D, D], FP8, isOutput=False)
    w2t = nc.declare_dram_parameter("w2t", [2 * D, D], FP8, isOutput=False)
    tp = nc.declare_dram_parameter("tp", [H, 128, 1024], BF16, isOutput=False)
    out = nc.declare_dram_parameter("out", [D, NQ], BF16, isOutput=True)

    xt_r = xt[:].rearrange("(s o p) f -> p s o f", p=128, s=2)   # [128,2,8,640]
    w1_r = w1t[:].rearrange("(s o p) f -> p s o f", p=128, s=2)  # [128,2,8,1024]
    w2_r = w2t[:].rearrange("(s o p) f -> p s o f", p=128, s=2)  # [128,2,8,1024]
    tp_r = tp[:].rearrange("h p f -> p h f")                     # [128, 8, 1024]

    with ExitStack() as ctx:
        e = ctx.enter_context
        xt_sb = e(nc.sbuf_tensor("xt_sb", [128, 2, 2, KT, LPAD], FP8))
        w1_sb = e(nc.sbuf_tensor("w1_sb", [128, 2, 2, KT, D], FP8))
        w2_sb = e(nc.sbuf_tensor("w2_sb", [128, 2, KT, D], FP8))
        TP0, TPW = (408, 240) if banded else (0, 1024)
        tp_sb = e(nc.sbuf_tensor("tp_sb", [128, H, TPW], BF16))
        tp_src = tp_r[:, :, TP0:TP0 + TPW]
        v_sb = e(nc.sbuf_tensor("v_sb", [128, LT, D], BF16))
        ab_sb = e(nc.sbuf_tensor("ab_sb", [128, H, NQ], BF16))
        zdum = e(nc.sbuf_tensor("zdum", [128, 384], BF16))
        ah_sb = e(nc.sbuf_tensor("ah_sb", [128, H, NQ], FP8))
        al_sb = e(nc.sbuf_tensor("al_sb", [128, H, NQ], FP8))
        o_sb = e(nc.sbuf_tensor("o_sb", [128, KT, NQ], BF16))
        ps = [e(nc.psum_tensor(f"ps{i}", [128, 512], F32)) for i in range(8)]

        sem_names = (["zd", "mmA", "mm1", "mm2", "mm3", "tp_d",
                      "cpA", "cpB", "c2hs", "c2as", "c2hv", "c2av", "c2lp",
                      "cp3v", "cp3s"]
                     + ["xt0a", "cpB4"]
                     + [f"xt_d{s}{c}b{p}" for s in (0, 1)
                        for c in range(K2T) for p in (0, 1)]
                     + [f"{n}s{s}b{p}" for n in ("w1a_d0", "w1a_g1", "w1a_g2",
                                                 "w1b")
                        for s in (0, 1) for p in (0, 1)]
                     + [f"w0_{n}s{s}" for n in ("d0", "g1", "g2", "b")
                        for s in (0, 1)]
                     + ["w2_s0", "w2_s1", "dmo7s"]
                     + [f"dmo{m}" for m in range(G3 - 1)])
        sems = {n: e(nc.semaphore(n)) for n in sem_names}

        WAVE_B_BANKS = [5, 6, 7, 0, 1]
        PH2_BANKS = [2, 3, 4, 5]
        # ph3 runs k2-outer over all 8 banks; ph2-shared banks (2,3,4,5)
        # sit at the tail so their readouts have time to free them
        PH3_BANKS = [0, 1, 6, 7, 2, 3, 4, 5]

        def cp3_waits(m, it):
            """(sem, count) pairs proving ph3 group m is fully copied out."""
            s = sems["cp3v" if m % 2 == 0 else "cp3s"]
            return [(s, it * 4 + m // 2 + 1)]

        def at_lo_wait(h, it):
            """(sem, count) proving head h's at_lo is written (pool)."""
            return sems["c2lp"], it * 8 + h + 1

        def at_hi_wait(h, it):
            """(sem, count) proving head h's at_hi is written."""
            if h % 2 == 0:
                return sems["c2hs"], it * 4 + h // 2 + 1
            return sems["c2hv"], it * 4 + h // 2 + 1

        def at_bank_wait(h, it):
            """(sem, count) proving head h's psum bank is fully read out
            (at_hi then at_bf on one engine; the at_bf sem is last)."""
            if h % 2 == 0:
                return sems["c2as"], it * 4 + h // 2 + 1
            return sems["c2av"], it * 4 + h // 2 + 1

        def w1_dmas(eng, it):
            # w1 DMAs for iteration `it`. Iter 0 issues from the pool
            # (SWDGE) and must use its own sems -- a semaphore cannot be
            # shared between SWDGE and HWDGE queues.
            buf = it % 2
            if it > 1:
                eng.wait_ge(sems["mm1"], (it - 1) * LT)
            it0_names = {"w1a_d0": "w0_d0", "w1a_g1": "w0_g1",
                         "w1a_g2": "w0_g2", "w1b": "w0_b"}

            def w1_dma(name, s, k0, k1, c0):
                sem = (sems[f"{it0_names[name]}s{s}"] if it == 0
                       else sems[f"{name}s{s}b{buf}"])
                eng.dma_start(
                    out=w1_sb[:, buf, s, k0:k1, c0:c0 + 512],
                    in_=w1_r[:, s, k0:k1, c0:c0 + 512],
                ).then_inc(sem, 16)

            w1_dma("w1a_d0", 0, 0, 2, 0)
            w1_dma("w1a_d0", 1, 0, 2, 0)
            w1_dma("w1a_g1", 0, 2, 4, 0)
            w1_dma("w1a_g1", 1, 2, 4, 0)
            w1_dma("w1a_g2", 0, 4, 8, 0)
            w1_dma("w1a_g2", 1, 4, 8, 0)
            w1_dma("w1b", 0, 0, 8, 512)
            w1_dma("w1b", 1, 0, 8, 512)

        def w2_dmas(eng, it):
            if it > 0:
                eng.wait_ge(sems["mm3"], it * G3)
            eng.dma_start(out=w2_sb[:, 0, :, :],
                          in_=w2_r[:, 0, :, :]).then_inc(sems["w2_s0"], 16)
            eng.dma_start(out=w2_sb[:, 1, :, :],
                          in_=w2_r[:, 1, :, :]).then_inc(sems["w2_s1"], 16)

        with nc.Block() as block:

            @block.sync
            def _(sync: bass.BassEngine):
                # xt chunks only; w1/w2 generate in parallel on the pool
                # queue so neither HWDGE generator falls behind in iter 0
                for it in range(iters):
                    buf = it % 2
                    if it > 1:
                        # xt buffer reuse: wave B (last reader) of iter it-2
                        sync.wait_ge(sems["mm1"], (it - 1) * LT)
                    for c in range(K2T):
                        for s in (0, 1):
                            if it == 0 and c == 0 and s == 0:
                                # iter 0: split so the first 4 l-tiles'
                                # columns land with a minimal transfer
                                sync.dma_start(
                                    out=xt_sb[:, 0, 0, 0:2, 0:512],
                                    in_=xt_r[:, 0, 0:2, 0:512],
                                ).then_inc(sems["xt0a"], 16)
                                sync.dma_start(
                                    out=xt_sb[:, 0, 0, 0:2, 512:LPAD],
                                    in_=xt_r[:, 0, 0:2, 512:LPAD],
                                ).then_inc(sems["xt_d00b0"], 16)
                                continue
                            if it == 0 and c == 0 and s == 1:
                                continue  # issued from the vector queue
                            sync.dma_start(
                                out=xt_sb[:, buf, s, 2 * c:2 * c + 2, :],
                                in_=xt_r[:, s, 2 * c:2 * c + 2, :],
                            ).then_inc(sems[f"xt_d{s}{c}b{buf}"], 16)
                    if it == 0:
                        sync.dma_start(out=tp_sb[:], in_=tp_src).then_inc(
                            sems["tp_d"], 16)
                    # w1 for the next iter: the sync HWDGE queue is idle for
                    # the rest of this iteration
                    if it + 1 < iters:
                        sync.wait_ge(sems["mm1"], it * LT)
                        w1_dmas(sync, it + 1)


            @block.tensor
            def _(tensor: bass.BassEngine):
                # p-state warmup: keep the PE busy from t~0 so the clock ramp
                # runs while the first DMAs land; sized to end just past
                # data-readiness (arriving early at the wait cluster costs a
                # ~1.6us deferred-dispatch cliff in the cost model)
                tensor.wait_ge(sems["zd"], 1)
                for n in WARMUP_NS:
                    tensor.matmul(ps[0][:, 0:n], zdum[:, 0:128],
                                  zdum[:, 128:128 + n], start=True, stop=True)

                def proj1_mms(bank, buf, k2, lt, c0):
                    mm = None
                    for ti, (sx, sw) in enumerate(TERMS):
                        mm = tensor.matmul(
                            bank[:, :],
                            xt_sb[:, buf, sx, 2 * k2:2 * k2 + 2,
                                  128 * lt:128 * lt + 128],
                            w1_sb[:, buf, sw, 2 * k2:2 * k2 + 2, c0:c0 + 512],
                            start=(k2 == 0 and ti == 0),
                            stop=(k2 == K2T - 1 and ti == len(TERMS) - 1),
                            perf_mode=DR,
                        )
                    return mm

                for it in range(iters):
                    buf = it % 2
                    # ---- wave A: v[:, 0:512], k-pair-outer, banks 0-4 ----
                    # cross-iter bank WAR: last users in iter it-1 were
                    # ph2 (banks 2,3,4 via h=4,5,6; bank 5 via h=7, last
                    # reader is the at_lo vector op) and ph3 (banks 6,7,0,1
                    # via m=4,5,6,7)
                    nth = (it // 2 + 1) * 16  # per-parity DMA count
                    for k2 in range(K2T):
                        if it == 0 and k2 == 0:
                            # startup: term-blocked so each mm-block needs
                            # only the DMA that could have landed by then
                            tensor.wait_ge(sems["xt0a"], 16)
                            tensor.wait_ge(sems["w0_d0s0"], 16)
                            for ti, (sx, sw) in enumerate(TERMS):
                                if ti == 1:
                                    tensor.wait_ge(sems["xt_d10b0"], 16)
                                elif ti == 2:
                                    tensor.wait_ge(sems["w0_d0s1"], 16)
                                for lt in range(LT):
                                    if ti == 0 and lt == 4:
                                        tensor.wait_ge(sems["xt_d00b0"], 16)
                                    tensor.matmul(
                                        ps[lt][:, :],
                                        xt_sb[:, 0, sx, 0:2,
                                              128 * lt:128 * lt + 128],
                                        w1_sb[:, 0, sw, 0:2, 0:512],
                                        start=(ti == 0), stop=False,
                                        perf_mode=DR,
                                    )
                            continue
                        tensor.wait_ge(sems[f"xt_d0{k2}b{buf}"], nth)
                        tensor.wait_ge(sems[f"xt_d1{k2}b{buf}"], nth)
                        def w1_wait(name, short):
                            if it == 0:
                                tensor.wait_ge(sems[f"w0_{short}s0"], 16)
                                tensor.wait_ge(sems[f"w0_{short}s1"], 16)
                            else:
                                nw = ((it + 1) // 2) * 16
                                tensor.wait_ge(sems[f"{name}s0b{buf}"], nw)
                                tensor.wait_ge(sems[f"{name}s1b{buf}"], nw)
                        if k2 == 0:
                            w1_wait("w1a_d0", "d0")
                        elif k2 == 1:
                            w1_wait("w1a_g1", "g1")
                        elif k2 == 2:
                            w1_wait("w1a_g2", "g2")
                        for lt in range(LT):
                            if k2 == 0 and it > 0:
                                # banks 0-4 <- ph3 m=0,1,4,5,6 copies (it-1)
                                for s, c in cp3_waits([0, 1, 4, 5, 6][lt],
                                                      it - 1):
                                    tensor.wait_ge(s, c)
                            mm = proj1_mms(ps[lt], buf, k2, lt, 0)
                            if k2 == K2T - 1:
                                mm.then_inc(sems["mmA"])
                    # ---- wave B: v[:, 512:1024], k-inner, banks [5,6,7,0,1] ----
                    for lt in range(LT):
                        bank = ps[WAVE_B_BANKS[lt]]
                        if lt < 3:
                            if it > 0:  # banks 5,6,7 <- ph3 m=7,2,3 (it-1)
                                for s, c in cp3_waits([7, 2, 3][lt], it - 1):
                                    tensor.wait_ge(s, c)
                        elif lt == 3:
                            # bank 0 <- wave A lt=0 copy of this iter
                            tensor.wait_ge(sems["cpA"], it * LT + 1)
                        else:
                            # bank 1 <- wave A lt=1 copy of this iter
                            tensor.wait_ge(sems["cpA"], it * LT + 2)
                        for k2 in range(K2T):
                            if lt == 0 and k2 == 0:
                                if it == 0:
                                    tensor.wait_ge(sems["w0_bs0"], 16)
                                    tensor.wait_ge(sems["w0_bs1"], 16)
                                else:
                                    nw = ((it + 1) // 2) * 16
                                    tensor.wait_ge(sems[f"w1bs0b{buf}"], nw)
                                    tensor.wait_ge(sems[f"w1bs1b{buf}"], nw)
                            mm = proj1_mms(bank, buf, k2, lt, 512)
                        mm.then_inc(sems["mm1"])

                    # ---- phase 2: attendedT per head (banded, bf16), banks 2-5 ----
                    if it == 0:
                        tensor.wait_ge(sems["tp_d"], 16)

                    def ph2_head(h):
                        bank = ps[PH2_BANKS[h % 4]]
                        # bank WAR: banks 2,3,4 <- wave A lt=2,3,4 copies;
                        # bank 5 <- wave B g=0 copy; h>=4 <- ph2 head h-4
                        # (the at_bf copy is the bank's last reader)
                        if h == 0:
                            tensor.wait_ge(sems["cpA"], it * LT + 3)
                        elif h == 1:
                            tensor.wait_ge(sems["cpA"], it * LT + 4)
                        elif h == 2:
                            tensor.wait_ge(sems["cpA"], it * LT + 5)
                        elif h == 3:
                            tensor.wait_ge(sems["cpB"], it * 4 + 1)
                        else:
                            s, c = at_bank_wait(h - 4, it)
                            tensor.wait_ge(s, c)
                        windows = attn_windows(h) if banded else [
                            (t, 0, NQ) for t in range(LT)]
                        for wi, (t, j0, j1) in enumerate(windows):
                            # data: v tile t, n-chunk h//4. Heads 1-3/5-7
                            # repeat head 0/4's thresholds -- skip the waits
                            if h == 0:
                                tensor.wait_ge(sems["cpA"], it * LT + t + 1)
                            elif h == 4 and t < 4:
                                tensor.wait_ge(sems["cpB"], it * 4 + t + 1)
                            elif h == 4:
                                tensor.wait_ge(sems["cpB4"], it + 1)
                            c0 = 512 - 128 * t + j0 - TP0
                            c1 = 512 - 128 * t + j1 - TP0
                            mm = tensor.matmul(
                                bank[:, j0:j1],
                                v_sb[:, t, 128 * h:128 * h + 128],
                                tp_sb[:, h, c0:c1],
                                start=(wi == 0), stop=(wi == len(windows) - 1),
                            )
                            if wi == len(windows) - 1:
                                mm.then_inc(sems["mm2"])

                    # ---- phase 3: outT = W2 @ attendedT (fp8) ----
                    # hybrid order: k2-outer for the first two passes (gives
                    # each head pair's readout a full pass of slack), then
                    # m-outer for k2=2,3 so the 8 psum groups close staggered
                    # and the copy/DMA drain pipeline overlaps the matmuls.
                    def ph3_mm(k2, m, ti, sa, sw):
                        at = ah_sb if sa == 0 else al_sb
                        return tensor.matmul(
                            ps[PH3_BANKS[m]][:, :],
                            w2_sb[:, sw, 2 * k2:2 * k2 + 2,
                                  128 * m:128 * m + 128],
                            at[:, 2 * k2:2 * k2 + 2, :],
                            start=(k2 == 0 and ti == 0),
                            stop=(k2 == K2T - 1 and ti == len(TERMS) - 1),
                            perf_mode=DR,
                        )

                    # ph2 heads 0-3 (v chunk 0 only)
                    for h in range(4):
                        ph2_head(h)
                    # ph3 k2=0 for m0-3: T1/T3 need only at_hi of heads 0,1;
                    # the T2 block (at_lo) follows once the pool sub lands
                    tensor.wait_ge(sems["w2_s0"], (it + 1) * 16)
                    tensor.wait_ge(sems["w2_s1"], (it + 1) * 16)
                    for s, c in (at_hi_wait(0, it), at_hi_wait(1, it)):
                        tensor.wait_ge(s, c)
                    for m in range(4):
                        # bank WAR: banks 0,1,6,7 <- wave B copies lt=3,4,1,2
                        if m == 1:
                            tensor.wait_ge(sems["cpB4"], it + 1)
                        else:
                            tensor.wait_ge(sems["cpB"],
                                           it * 4 + [4, 0, 2, 3][m])
                        for ti in (0, 2):
                            ph3_mm(0, m, ti, *TERMS[ti])
                    # ph3 T1/T3 (at_hi only) for m0-3 at k2=1
                    for s, c in (at_hi_wait(2, it), at_hi_wait(3, it)):
                        tensor.wait_ge(s, c)
                    for m in range(4):
                        for ti in (0, 2):
                            ph3_mm(1, m, ti, *TERMS[ti])
                    # ph2 heads 4-7 (banks freed by heads 0-3 readouts)
                    for h in range(4, G2):
                        ph2_head(h)
                    # deferred T2 (at_lo) terms for m0-3 -- placed here so
                    # the h4-7 readouts have time to land before ph3b
                    for h in (0, 1, 2, 3):
                        s, c = at_lo_wait(h, it)
                        tensor.wait_ge(s, c)
                    for k2 in (0, 1):
                        for m in range(4):
                            ph3_mm(k2, m, 1, *TERMS[1])
                    # ph3 m4-7: full k2=0 triplets, then k2=1 triplets
                    for m in range(4, G3):
                        s, c = at_bank_wait(m, it)
                        tensor.wait_ge(s, c)
                        for ti, (sa, sw) in enumerate(TERMS):
                            ph3_mm(0, m, ti, sa, sw)
                    for m in range(4, G3):
                        for ti, (sa, sw) in enumerate(TERMS):
                            ph3_mm(1, m, ti, sa, sw)
                    # drain: m-outer k2=2,3; groups close staggered so the
                    # copy/DMA pipeline overlaps the matmuls
                    for h in (4, 5, 6, 7):
                        s, c = at_lo_wait(h, it)
                        tensor.wait_ge(s, c)
                    for m in range(G3):
                        for k2 in (2, 3):
                            for ti, (sa, sw) in enumerate(TERMS):
                                mm = ph3_mm(k2, m, ti, sa, sw)
                        mm.then_inc(sems["mm3"])

            @block.vector
            def _(vector: bass.BassEngine):
                for it in range(iters):
                    # wave A copies: v[:, lt, 0:512], banks 0-4, after last MM
                    for lt in range(LT):
                        vector.wait_ge(sems["mmA"], it * LT + lt + 1)
                        vector.tensor_copy(
                            out=v_sb[:, lt, 0:512], in_=ps[lt][:, :],
                        ).then_inc(sems["cpA"])
                    # wave B lt=4 copy (the pool cannot read psum)
                    vector.wait_ge(sems["mm1"], it * LT + 5)
                    vector.tensor_copy(
                        out=v_sb[:, 4, 512:1024],
                        in_=ps[WAVE_B_BANKS[4]][:, :],
                    ).then_inc(sems["cpB4"])
                    # at_hi / at_bf for odd heads (ah h1/h3 front-loaded)
                    def v_ah(h):
                        vector.tensor_copy(
                            out=ah_sb[:, h, :], in_=ps[PH2_BANKS[h % 4]][:, :],
                        ).then_inc(sems["c2hv"])

                    def v_ab(h):
                        vector.tensor_copy(
                            out=ab_sb[:, h, :], in_=ps[PH2_BANKS[h % 4]][:, :],
                        ).then_inc(sems["c2av"])

                    vector.wait_ge(sems["mm2"], it * G2 + 2)
                    v_ah(1)
                    vector.wait_ge(sems["mm2"], it * G2 + 4)
                    v_ah(3)
                    v_ab(1)
                    v_ab(3)
                    vector.wait_ge(sems["mm2"], it * G2 + 6)
                    v_ah(5)
                    v_ab(5)
                    vector.wait_ge(sems["mm2"], it * G2 + 8)
                    v_ah(7)
                    v_ab(7)
                    for m in [0, 2, 4, 6]:
                        vector.wait_ge(sems["mm3"], it * G3 + m + 1)
                        if it > 0:
                            vector.wait_ge(sems[f"dmo{m}"], it * 16)
                        vector.tensor_copy(
                            out=o_sb[:, m, :],
                            in_=ps[PH3_BANKS[m]][:, :],
                        ).then_inc(sems["cp3v"])


            @block.gpsimd
            def _(gpsimd: bass.BassEngine):
                # zero the PE-warmup tile before anything else (fast on pool)
                gpsimd.memset(zdum[:], 0).then_inc(sems["zd"])
                w1_dmas(gpsimd, 0)
                w2_dmas(gpsimd, 0)
                for it in range(iters):
                    # at_lo = fp8(at_bf - at_hi), SBUF-only (pool cannot
                    # access psum on hardware)
                    for h in range(G2):
                        s, c = at_bank_wait(h, it)
                        gpsimd.wait_ge(s, c)
                        gpsimd.tensor_sub(
                            al_sb[:, h, :], ab_sb[:, h, :], ah_sb[:, h, :],
                        ).then_inc(sems["c2lp"])
                    # ---- output DMAs (m7 goes out on the scalar HWDGE) ----
                    for m in range(G3 - 1):
                        for s, c in cp3_waits(m, it):
                            gpsimd.wait_ge(s, c)
                        gpsimd.dma_start(
                            out=out[128 * m:128 * m + 128, :],
                            in_=o_sb[:, m, :],
                        ).then_inc(sems[f"dmo{m}"], 16)
                    # w2 for the next iter: deadline is its ph3, so after
                    # the outs is fine (mm3 guard is already satisfied here)
                    if it + 1 < iters:
                        w2_dmas(gpsimd, it + 1)
                for m in range(G3 - 1):
                    gpsimd.wait_ge(sems[f"dmo{m}"], iters * 16)
                gpsimd.wait_ge(sems["dmo7s"], iters * 16)

            @block.scalar
            def _(scalar: bass.BassEngine):
                # iter 0's xt lo-half k-pair 0: separate HWDGE queue so it
                # is not behind the hi-half chunks on the sync queue
                scalar.dma_start(
                    out=xt_sb[:, 0, 1, 0:2, :], in_=xt_r[:, 1, 0:2, :],
                ).then_inc(sems["xt_d10b0"], 16)
                for it in range(iters):
                    # wave B copies lt 0-3 (lt 4 runs on the pool so the
                    # at_hi chain isn't stuck behind it)
                    for lt in range(4):
                        scalar.wait_ge(sems["mm1"], it * LT + lt + 1)
                        scalar.copy(v_sb[:, lt, 512:1024],
                                    ps[WAVE_B_BANKS[lt]][:, :]).then_inc(sems["cpB"])
                    # at_hi (fp8) / at_bf (bf16) for even heads; the pool
                    # derives at_lo = at_bf - at_hi off the psum path.
                    # ah h0/h2 are front-loaded: they gate ph3's T1/T3.
                    def s_ah(h):
                        scalar.copy(ah_sb[:, h, :],
                                    ps[PH2_BANKS[h % 4]][:, :]).then_inc(
                            sems["c2hs"])

                    def s_ab(h):
                        scalar.copy(ab_sb[:, h, :],
                                    ps[PH2_BANKS[h % 4]][:, :]).then_inc(
                            sems["c2as"])

                    scalar.wait_ge(sems["mm2"], it * G2 + 1)
                    s_ah(0)
                    scalar.wait_ge(sems["mm2"], it * G2 + 3)
                    s_ah(2)
                    s_ab(0)
                    s_ab(2)
                    scalar.wait_ge(sems["mm2"], it * G2 + 5)
                    s_ah(4)
                    s_ab(4)
                    scalar.wait_ge(sems["mm2"], it * G2 + 7)
                    s_ah(6)
                    s_ab(6)
                    for m in [1, 3, 5, 7]:
                        scalar.wait_ge(sems["mm3"], it * G3 + m + 1)
                        if it > 0:
                            scalar.wait_ge(
                                sems["dmo7s" if m == 7 else f"dmo{m}"],
                                it * 16)
                        scalar.copy(o_sb[:, m, :],
                                    ps[PH3_BANKS[m]][:, :]).then_inc(
                            sems["cp3s"])
                        if m == 7:
                            # m7 is the tail-critical group: its output DMA
                            # goes out on this (otherwise idle) HWDGE queue
                            # instead of the pool's SWDGE, whose end-of-
                            # kernel drain would add ~0.5us to the barrier
                            scalar.dma_start(
                                out=out[896:1024, :],
                                in_=o_sb[:, 7, :],
                            ).then_inc(sems["dmo7s"], 16)


    return nc


# ---------------- host side ----------------

_GRAPH_CACHE: dict = {}


def get_graph(iters: int = 1, banded: bool = True) -> bass.Bass:
    key = (iters, banded)
    if key not in _GRAPH_CACHE:
        _GRAPH_CACHE[key] = build_graph(iters, banded)
    return _GRAPH_CACHE[key]


class Runner:
    """Compile-once executor for one Bass graph across the 8 cores.

    Mirrors bass2jax.run_bass_via_pjrt but keeps the jitted callable so
    repeated invocations don't re-trace/re-compile.
    """

    def __init__(self, nc: bass.Bass, n_cores: int = N_CORES):
        import jax
        from jax.sharding import Mesh, PartitionSpec
        from jax.experimental.shard_map import shard_map
        from concourse import bass2jax, mybir as _mb

        bass2jax.install_neuronx_cc_hook()
        self.n_cores = n_cores

        partition_name = (nc.partition_id_tensor.name
                          if nc.partition_id_tensor else None)
        in_names, out_names, out_avals, zero_shapes = [], [], [], []
        for alloc in nc.m.functions[0].allocations:
            if not isinstance(alloc, _mb.MemoryLocationSet):
                continue
            name = alloc.memorylocations[0].name
            if alloc.kind == "ExternalInput":
                if name != partition_name:
                    in_names.append(name)
            elif alloc.kind == "ExternalOutput":
                out_names.append(name)
                shape = tuple(alloc.tensor_shape)
                dtype = _mb.dt.np(alloc.dtype)
                out_avals.append(jax.core.ShapedArray(shape, dtype))
                zero_shapes.append((shape, dtype))
        self.in_names = list(in_names)
        self.out_names = out_names
        self.out_avals = out_avals
        self.zero_shapes = zero_shapes
        n_params = len(in_names)
        all_names = in_names + out_names
        if partition_name is not None:
            all_names = all_names + [partition_name]

        def _body(*args):
            operands = list(args)
            if partition_name is not None:
                operands.append(bass2jax.partition_id_tensor())
            outs = bass2jax._bass_exec_p.bind(
                *operands,
                out_avals=tuple(out_avals),
                in_names=tuple(all_names),
                out_names=tuple(out_names),
                lowering_input_output_aliases=(),
                sim_require_finite=True,
                sim_require_nnan=True,
                nc=nc,
            )
            return tuple(outs)

        devices = jax.devices()[:n_cores]
        mesh = Mesh(np.asarray(devices), ("core",))
        self._mesh = mesh
        n_outs = len(out_names)
        self._fn = jax.jit(
            shard_map(_body, mesh=mesh,
                      in_specs=(PartitionSpec("core"),) * (n_params + n_outs),
                      out_specs=(PartitionSpec("core"),) * n_outs,
                      check_rep=False),
            donate_argnums=tuple(range(n_params, n_params + n_outs)),
            keep_unused=True,
        )

    def stage(self, in_maps):
        """device_put the concatenated inputs once; returns device arrays."""
        import jax
        concat_in = [
            np.concatenate([np.asarray(m[name]) for m in in_maps], axis=0)
            for name in self.in_names
        ]
        return [jax.device_put(a) for a in concat_in]

    def make_zeros(self):
        if not hasattr(self, "_zeros_fn"):
            import jax
            import jax.numpy as jnp
            from jax.sharding import NamedSharding, PartitionSpec
            shardings = tuple(
                NamedSharding(self._mesh, PartitionSpec("core"))
                for _ in self.zero_shapes)
            shapes = [((self.n_cores * s[0], *s[1:]), d)
                      for s, d in self.zero_shapes]

            def _mk():
                return tuple(jnp.zeros(sh, dt) for sh, dt in shapes)

            self._zeros_fn = jax.jit(_mk, out_shardings=shardings)
        return list(self._zeros_fn())

    def run_staged(self, dev_in, dev_zeros):
        return self._fn(*dev_in, *dev_zeros)

    def __call__(self, in_maps):
        out_arrs = self._fn(*self.stage(in_maps), *self.make_zeros())
        return [
            {name: np.asarray(out_arrs[i]).reshape(
                self.n_cores, *self.out_avals[i].shape)[c]
             for i, name in enumerate(self.out_names)}
            for c in range(self.n_cores)
        ]


_RUNNER_CACHE: dict = {}


def get_runner(iters: int = 1) -> "Runner":
    if iters not in _RUNNER_CACHE:
        _RUNNER_CACHE[iters] = Runner(get_graph(iters))
    return _RUNNER_CACHE[iters]


def _hilo(a: np.ndarray) -> tuple[np.ndarray, np.ndarray]:
    """fp8 hi/lo split: a ~= hi + lo with hi = fp8(a), lo = fp8(a - hi)."""
    hi = a.astype(NP_FP8)
    lo = (a - hi.astype(np.float32)).astype(NP_FP8)
    return hi, lo


def make_in_maps(values: np.ndarray, input_weights: np.ndarray,
                 output_weight: np.ndarray) -> list:
    w1h, w1l = _hilo(WSCALE * input_weights.T.astype(np.float32))
    w1t = np.concatenate([w1h, w1l], axis=0)          # [2048, 1024] fp8
    w2h, w2l = _hilo(WSCALE * output_weight.T.astype(np.float32))
    w2t = np.concatenate([w2h, w2l], axis=0)
    tpt = gauss_toeplitz_table()
    in_maps = []
    for core in range(N_CORES):
        b, c = divmod(core, 4)
        lo, hi = c * CHUNK - HALO_L, c * CHUNK + CHUNK + HALO_R
        src_lo, src_hi = max(lo, 0), min(hi, L)
        xt_pad = np.zeros((2 * D, LPAD), dtype=NP_FP8)
        xs = values[b, src_lo:src_hi, :].T.astype(np.float32)
        xh, xl = _hilo(xs)
        xt_pad[0:D, src_lo - lo:src_hi - lo] = xh
        xt_pad[D:2 * D, src_lo - lo:src_hi - lo] = xl
        in_maps.append({"xt": xt_pad, "w1t": w1t, "w2t": w2t, "tp": tpt})
    return in_maps


def assemble(results: list) -> np.ndarray:
    out = np.empty((B, L, D), dtype=np.float32)
    inv = np.float32(1.0 / WSCALE)
    for core in range(N_CORES):
        b, c = divmod(core, 4)
        out[b, c * CHUNK:(c + 1) * CHUNK, :] = \
            results[core]["out"].T.astype(np.float32) * inv
    return out


def kernel(values: np.ndarray, input_weights: np.ndarray,
           output_weight: np.ndarray) -> np.ndarray:
    in_maps = make_in_maps(values, input_weights, output_weight)
    try:
        return assemble(get_runner(1)(in_maps))
    except Exception:
        # fallback: canonical SPMD path (re-traces per call but always works)
        res = run_bass_kernel_spmd(get_graph(1), in_maps,
                                   core_ids=list(range(N_CORES)))
        return assemble(res.results)


# revision 124
# speedup vs baseline: 1.2471x; 1.0124x over previous
"""Trainium2 Bass kernel for nn_Attention (Gaussian banded attention).

Math (reference):
    v = values @ input_weights.T                      # [B,L,D]
    probs[h,q,k] = N(k - q - off_h; std_h)            # Gaussian, depends on k-q only
    attended[b,h,q,:] = sum_k probs[h,q,k] v[b,k,h*pd:(h+1)*pd]
    out = attended_merged @ output_weight.T           # [B,L,D]

Key structural facts exploited:
  - probs is a banded Toeplitz matrix per head: nonzero only for
    k - q in [off - 6*std, off + 6*std] (6-sigma truncation, error ~1e-8).
    Widest band: std=8, off=-8 -> k-q in [-56, 40].
  - So attention is a narrow depthwise convolution along L; no [L,L] matmul.
  - Batch x L sharding is embarrassingly parallel given a halo of
    56 backward / 40 forward rows of the INPUT (v is a row-wise projection,
    zero rows project to zero since there is no bias).

Precision/speed scheme: both dense projections run on the PE in fp8e4
(e4m3) with perf_mode=DoubleRow (K=256 per instruction at 0.5 cycles per
output column -> 4x the bf16 FLOP rate), using a 3-term error-compensated
expansion per projection:
    X @ W  ~=  X_hi@W_hi + X_lo@W_hi + X_hi@W_lo
with X = X_hi + X_lo (hi = fp8(X), lo = fp8(X - X_hi)) and W shipped
pre-scaled by 64 (w values ~0.02 would be fp8-subnormal unscaled). The
64x output scale is absorbed into the Gaussian table (tp/64) after
attention, and divided out on the host for the final output. Attention
itself stays bf16 (small cost). Measured end-to-end rel err ~4e-3, same
as the all-bf16 variant.

Sharding: 8 cores = (B=2) x (4 chunks of 512 rows of L). Each core gets
x.T as stacked fp8 hi/lo [2048, 640] (56 halo + 512 + 40 halo + 32 zero
pad), computes in [D, L]->[L, D]->[D, L] layouts, and writes out.T
[1024, 512] bf16 carrying a 64x scale (host casts/divides on
reassembly). No collectives.

Schedule highlights (see build_graph):
  - attended readout: per head, one engine (scalar evens / vector odds)
    copies psum -> at_hi (fp8) then psum -> at_bf (bf16); the pool derives
    at_lo = fp8(at_bf - at_hi) in SBUF (the pool cannot access psum).
  - ph3 interleaves with ph2: the at_hi-only terms (T1/T3) for output
    tiles m0-3 run between ph2 head groups, so no psum bank or readout is
    ever awaited for long; groups m0-7 close staggered in a final m-outer
    k2=2,3 drain that overlaps the copy/DMA pipeline.
  - DMA generation is spread over three queues: sync (xt + next-iter w1),
    pool/SWDGE (iter-0 weights, w2, outputs m0-6), scalar (iter-0 xt lo
    half + the tail-critical m7 output, dodging the SWDGE end drain).
  - warmup matmuls keep the PE busy from t~0 (the cost model's p-state
    ramp is wall-clock based and a cold wait defers dispatch ~1.6us).

Cost-model performance: 31,343 ns single-shot; 26,134 ns/iter steady
state (PE ~99% busy; bf16 floor would be ~33,000 ns).
"""

import math
from contextlib import ExitStack

import numpy as np
import ml_dtypes

import concourse.bass as bass
from concourse import mybir
from concourse.bass_utils import run_bass_kernel_spmd

# ---- NEFF disk cache (keyed by BIR hash) to avoid recompiling identical
# graphs in fresh processes ----
import hashlib
import os
import shutil

_NEFF_CACHE_DIR = os.environ.get("NEFF_CACHE_DIR", "/root/neff_cache")


def _install_neff_cache():
    import concourse.bass_utils as _bu
    import concourse.bass2jax as _b2j
    if getattr(_bu, "_neff_cache_installed", False):
        return
    orig = _bu.compile_bir_kernel

    def cached(bir_json, tmpdir, neff_name="file.neff"):
        cpath = None
        try:
            os.makedirs(_NEFF_CACHE_DIR, exist_ok=True)
            key = hashlib.sha256(bir_json).hexdigest()[:32]
            cpath = os.path.join(_NEFF_CACHE_DIR, f"{key}.neff")
            dst = os.path.join(tmpdir, neff_name)
            if os.path.exists(cpath):
                shutil.copy(cpath, dst)
                return dst
        except OSError:
            cpath = None  # cache unusable; plain compile below
        path = orig(bir_json, tmpdir, neff_name)
        if cpath is not None:
            try:
                shutil.copy(path, cpath)
            except OSError:
                pass
        return path

    _bu.compile_bir_kernel = cached
    _b2j.compile_bir_kernel = cached
    _bu._neff_cache_installed = True


_install_neff_cache()

# ---------------- problem constants (hardcoded per spec) ----------------
B, L, D = 2, 2048, 1024
H, PD = 8, 128
ATTN_STD = np.array([1.0, 2.0, 4.0, 8.0, 1.0, 2.0, 4.0, 8.0], dtype=np.float64)
ATTN_OFFSET = np.array([-1.0, -2.0, -4.0, -8.0, -1.0, -2.0, -4.0, -8.0], dtype=np.float64)

N_CORES = 8
CHUNK = 512            # output rows per core
HALO_L, HALO_R = 56, 40
LPAD = 640             # 56 + 512 + 40 = 608, padded to 5*128
LT = 5                 # l-tiles of v (640 / 128)
KT = 8                 # d tiles (1024 / 128)
K2T = 4                # DoubleRow k-pair tiles (1024 / 256)
NQ = CHUNK             # query columns per core

BF16 = mybir.dt.bfloat16
F32 = mybir.dt.float32
FP8 = mybir.dt.float8e4
NP_FP8 = ml_dtypes.float8_e4m3
WSCALE = 64.0          # fp8 weights are shipped as 64*W

G2 = H                 # attention heads -> attendedT
G3 = KT                # proj2 d_out tiles -> outT
DR = mybir.MatmulPerfMode.DoubleRow

# (x-split, w-split) term order for the 3-term compensated product:
# (hi,hi), (lo,hi), (hi,lo) -- s index 0=hi, 1=lo
TERMS = ((0, 0), (1, 0), (0, 1))

# warmup matmul N sizes (mid-pstate: ~0.833 ns per column), tuned so the
# last one ends just past first-data-readiness (~920ns)
WARMUP_NS = (256, 224)


def gauss_toeplitz_table() -> np.ndarray:
    """tp[h, r, m] = g_h(r - (m - 512) - 56) / 64, shape [H, 128, 1024] bf16.

    For v-tile t (rows k' = 128t + r of padded-local v) the attention rhs is
    tp[h][:, 512-128t : 1024-128t] so that rhs[r, q'] = g_h(128t + r - q' - 56),
    which is probs[h, q, k].T in padded-local coordinates. The /64 cancels
    the 64x scale carried by v (fp8 weights are shipped as 64*W1^T).
    """
    r = np.arange(128, dtype=np.float64)[:, None]
    m = np.arange(1024, dtype=np.float64)[None, :]
    delta = r - (m - 512.0) - 56.0  # = k - q
    tables = []
    for h in range(H):
        std, off = ATTN_STD[h], ATTN_OFFSET[h]
        z = (delta - off) / std
        g = np.exp(-0.5 * z * z) / (std * math.sqrt(2.0 * math.pi))
        g[np.abs(z) > 6.0] = 0.0
        tables.append(g / WSCALE)
    return np.stack(tables).astype(ml_dtypes.bfloat16)


def attn_windows(h: int):
    """Static (t, j0, j1) list: nonzero q-column window of v-tile t for head h,
    8-aligned. Coverage of [0,512) is guaranteed (window width > 128).
    5-sigma truncation (leaked mass ~3e-6; the tp table is zeroed at 6)."""
    std, off = int(ATTN_STD[h]), int(ATTN_OFFSET[h])
    wlo = -56 - off - 5 * std
    whi = 71 - off + 5 * std
    res = []
    for t in range(LT):
        j0 = max(0, 128 * t + wlo)
        j1 = min(NQ, 128 * t + whi + 1)
        if j0 >= j1:
            continue
        j0 = (j0 // 8) * 8
        j1 = min(NQ, ((j1 + 7) // 8) * 8)
        res.append((t, j0, j1))
    return res


def build_graph(iters: int = 1, banded: bool = True) -> bass.Bass:
    """One SPMD core program. iters>1 repeats the whole kernel (including
    DMAs) with monotonically increasing semaphore thresholds, for timing.

    Phase structure per iteration (PE program order):
      warmup: discarded matmuls on a zeroed tile during the first DMA's
              latency window (p-state ramp off the critical path);
      wave A: v[:, 0:512]  = x @ W1a  -- k-pair-outer over psum banks 0-4 so
              the PE streams while the xt/w1a DMAs arrive; 3 fp8 DoubleRow
              terms per (k-pair, l-tile);
      wave B: v[:, 512:1024] = x @ W1b -- k-inner, data resident, banks [5,6,7,0,1];
      ph2/ph3 interleaved:
        ph2 h0-3 (banks 2-5, bf16 banded windows) -> ph3 T1/T3 for m0-3 at
        k2=0,1 (banks 0,1,6,7; needs only at_hi of h0-3) -> ph2 h4-7 ->
        deferred ph3 T2 terms for m0-3 -> ph3 m4-7 (k2=0,1; banks 2-5) ->
        drain: m-outer k2=2,3, all 8 groups close staggered.
    Readout per head: at_hi (fp8) + at_bf (bf16) psum copies on one engine
    (scalar: even heads, vector: odd), at_lo = fp8(at_bf - at_hi) on the
    pool in SBUF. Wave copies: A -> vector, B lt0-3 -> scalar, B lt4 ->
    vector; ph3 o_sb copies alternate engines; xt/w1 double-buffered so
    iterations pipeline.
    """
    nc = bass.Bass()

    xt = nc.declare_dram_parameter("xt", [2 * D, LPAD], FP8, isOutput=False)
    w1t = nc.declare_dram_parameter("w1t", [2 * D, D], FP8, isOutput=False)
    w2t = nc.declare_dram_parameter("w2t", [2 * D, D], FP8, isOutput=False)
    tp = nc.declare_dram_parameter("tp", [H, 128, 1024], BF16, isOutput=False)
    out = nc.declare_dram_parameter("out", [D, NQ], BF16, isOutput=True)

    xt_r = xt[:].rearrange("(s o p) f -> p s o f", p=128, s=2)   # [128,2,8,640]
    w1_r = w1t[:].rearrange("(s o p) f -> p s o f", p=128, s=2)  # [128,2,8,1024]
    w2_r = w2t[:].rearrange("(s o p) f -> p s o f", p=128, s=2)  # [128,2,8,1024]
    tp_r = tp[:].rearrange("h p f -> p h f")                     # [128, 8, 1024]

    with ExitStack() as ctx:
        e = ctx.enter_context
        xt_sb = e(nc.sbuf_tensor("xt_sb", [128, 2, 2, KT, LPAD], FP8))
        w1_sb = e(nc.sbuf_tensor("w1_sb", [128, 2, 2, KT, D], FP8))
        w2_sb = e(nc.sbuf_tensor("w2_sb", [128, 2, KT, D], FP8))
        TP0, TPW = (408, 240) if banded else (0, 1024)
        tp_sb = e(nc.sbuf_tensor("tp_sb", [128, H, TPW], BF16))
        tp_src = tp_r[:, :, TP0:TP0 + TPW]
        v_sb = e(nc.sbuf_tensor("v_sb", [128, LT, D], BF16))
        ab_sb = e(nc.sbuf_tensor("ab_sb", [128, H, NQ], BF16))
        zdum = e(nc.sbuf_tensor("zdum", [128, 384], BF16))
        ah_sb = e(nc.sbuf_tensor("ah_sb", [128, H, NQ], FP8))
        al_sb = e(nc.sbuf_tensor("al_sb", [128, H, NQ], FP8))
        o_sb = e(nc.sbuf_tensor("o_sb", [128, KT, NQ], BF16))
        ps = [e(nc.psum_tensor(f"ps{i}", [128, 512], F32)) for i in range(8)]

        sem_names = (["zd", "mmA", "mm1", "mm2", "mm3", "tp_d",
                      "cpA", "cpB", "c2hs", "c2as", "c2hv", "c2av", "c2lp",
                      "cp3v", "cp3s"]
                     + ["xt0a", "cpB4"]
                     + [f"xt_d{s}{c}b{p}" for s in (0, 1)
                        for c in range(K2T) for p in (0, 1)]
                     + [f"{n}s{s}b{p}" for n in ("w1a_d0", "w1a_g1", "w1a_g2",
                                                 "w1b")
                        for s in (0, 1) for p in (0, 1)]
                     + [f"w0_{n}s{s}" for n in ("d0", "g1", "g2", "b")
                        for s in (0, 1)]
                     + ["w2_s0", "w2_s1", "dmo7s"]
                     + [f"dmo{m}" for m in range(G3 - 1)])
        sems = {n: e(nc.semaphore(n)) for n in sem_names}

        WAVE_B_BANKS = [5, 6, 7, 0, 1]
        PH2_BANKS = [2, 3, 4, 5]
        # ph3 runs k2-outer over all 8 banks; ph2-shared banks (2,3,4,5)
        # sit at the tail so their readouts have time to free them
        PH3_BANKS = [0, 1, 6, 7, 2, 3, 4, 5]

        def cp3_waits(m, it):
            """(sem, count) pairs proving ph3 group m is fully copied out."""
            s = sems["cp3v" if m % 2 == 0 else "cp3s"]
            return [(s, it * 4 + m // 2 + 1)]

        def at_lo_wait(h, it):
            """(sem, count) proving head h's at_lo is written (pool)."""
            return sems["c2lp"], it * 8 + h + 1

        def at_hi_wait(h, it):
            """(sem, count) proving head h's at_hi is written."""
            if h % 2 == 0:
                return sems["c2hs"], it * 4 + h // 2 + 1
            return sems["c2hv"], it * 4 + h // 2 + 1

        def at_bank_wait(h, it):
            """(sem, count) proving head h's psum bank is fully read out
            (at_hi then at_bf on one engine; the at_bf sem is last)."""
            if h % 2 == 0:
                return sems["c2as"], it * 4 + h // 2 + 1
            return sems["c2av"], it * 4 + h // 2 + 1

        def w1_dmas(eng, it):
            # w1 DMAs for iteration `it`. Iter 0 issues from the pool
            # (SWDGE) and must use its own sems -- a semaphore cannot be
            # shared between SWDGE and HWDGE queues.
            buf = it % 2
            if it > 1:
                eng.wait_ge(sems["mm1"], (it - 1) * LT)
            it0_names = {"w1a_d0": "w0_d0", "w1a_g1": "w0_g1",
                         "w1a_g2": "w0_g2", "w1b": "w0_b"}

            def w1_dma(name, s, k0, k1, c0):
                sem = (sems[f"{it0_names[name]}s{s}"] if it == 0
                       else sems[f"{name}s{s}b{buf}"])
                eng.dma_start(
                    out=w1_sb[:, buf, s, k0:k1, c0:c0 + 512],
                    in_=w1_r[:, s, k0:k1, c0:c0 + 512],
                ).then_inc(sem, 16)

            w1_dma("w1a_d0", 0, 0, 2, 0)
            w1_dma("w1a_d0", 1, 0, 2, 0)
            w1_dma("w1a_g1", 0, 2, 4, 0)
            w1_dma("w1a_g1", 1, 2, 4, 0)
            w1_dma("w1a_g2", 0, 4, 8, 0)
            w1_dma("w1a_g2", 1, 4, 8, 0)
            w1_dma("w1b", 0, 0, 8, 512)
            w1_dma("w1b", 1, 0, 8, 512)

        def w2_dmas(eng, it):
            if it > 0:
                eng.wait_ge(sems["mm3"], it * G3)
            eng.dma_start(out=w2_sb[:, 0, :, :],
                          in_=w2_r[:, 0, :, :]).then_inc(sems["w2_s0"], 16)
            eng.dma_start(out=w2_sb[:, 1, :, :],
                          in_=w2_r[:, 1, :, :]).then_inc(sems["w2_s1"], 16)

        with nc.Block() as block:

            @block.sync
            def _(sync: bass.BassEngine):
                # xt chunks only; w1/w2 generate in parallel on the pool
                # queue so neither HWDGE generator falls behind in iter 0
                for it in range(iters):
                    buf = it % 2
                    if it > 1:
                        # xt buffer reuse: wave B (last reader) of iter it-2
                        sync.wait_ge(sems["mm1"], (it - 1) * LT)
                    for c in range(K2T):
                        for s in (0, 1):
                            if it == 0 and c == 0 and s == 0:
                                # iter 0: split so the first 4 l-tiles'
                                # columns land with a minimal transfer
                                sync.dma_start(
                                    out=xt_sb[:, 0, 0, 0:2, 0:512],
                                    in_=xt_r[:, 0, 0:2, 0:512],
                                ).then_inc(sems["xt0a"], 16)
                                sync.dma_start(
                                    out=xt_sb[:, 0, 0, 0:2, 512:LPAD],
                                    in_=xt_r[:, 0, 0:2, 512:LPAD],
                                ).then_inc(sems["xt_d00b0"], 16)
                                continue
                            if it == 0 and c == 0 and s == 1:
                                continue  # issued from the vector queue
                            sync.dma_start(
                                out=xt_sb[:, buf, s, 2 * c:2 * c + 2, :],
                                in_=xt_r[:, s, 2 * c:2 * c + 2, :],
                            ).then_inc(sems[f"xt_d{s}{c}b{buf}"], 16)
                    if it == 0:
                        sync.dma_start(out=tp_sb[:], in_=tp_src).then_inc(
                            sems["tp_d"], 16)
                    # w1 for the next iter: the sync HWDGE queue is idle for
                    # the rest of this iteration
                    if it + 1 < iters:
                        sync.wait_ge(sems["mm1"], it * LT)
                        w1_dmas(sync, it + 1)


            @block.tensor
            def _(tensor: bass.BassEngine):
                # p-state warmup: keep the PE busy from t~0 so the clock ramp
                # runs while the first DMAs land; sized to end just past
                # data-readiness (arriving early at the wait cluster costs a
                # ~1.6us deferred-dispatch cliff in the cost model)
                tensor.wait_ge(sems["zd"], 1)
                for n in WARMUP_NS:
                    tensor.matmul(ps[0][:, 0:n], zdum[:, 0:128],
                                  zdum[:, 128:128 + n], start=True, stop=True)

                def proj1_mms(bank, buf, k2, lt, c0):
                    mm = None
                    for ti, (sx, sw) in enumerate(TERMS):
                        mm = tensor.matmul(
                            bank[:, :],
                            xt_sb[:, buf, sx, 2 * k2:2 * k2 + 2,
                                  128 * lt:128 * lt + 128],
                            w1_sb[:, buf, sw, 2 * k2:2 * k2 + 2, c0:c0 + 512],
                            start=(k2 == 0 and ti == 0),
                            stop=(k2 == K2T - 1 and ti == len(TERMS) - 1),
                            perf_mode=DR,
                        )
                    return mm

                for it in range(iters):
                    buf = it % 2
                    # ---- wave A: v[:, 0:512], k-pair-outer, banks 0-4 ----
                    # cross-iter bank WAR: last users in iter it-1 were
                    # ph2 (banks 2,3,4 via h=4,5,6; bank 5 via h=7, last
                    # reader is the at_lo vector op) and ph3 (banks 6,7,0,1
                    # via m=4,5,6,7)
                    nth = (it // 2 + 1) * 16  # per-parity DMA count
                    for k2 in range(K2T):
                        if it == 0 and k2 == 0:
                            # startup: term-blocked so each mm-block needs
                            # only the DMA that could have landed by then
                            tensor.wait_ge(sems["xt0a"], 16)
                            tensor.wait_ge(sems["w0_d0s0"], 16)
                            for ti, (sx, sw) in enumerate(TERMS):
                                if ti == 1:
                                    tensor.wait_ge(sems["xt_d10b0"], 16)
                                elif ti == 2:
                                    tensor.wait_ge(sems["w0_d0s1"], 16)
                                for lt in range(LT):
                                    if ti == 0 and lt == 4:
                                        tensor.wait_ge(sems["xt_d00b0"], 16)
                                    tensor.matmul(
                                        ps[lt][:, :],
                                        xt_sb[:, 0, sx, 0:2,
                                              128 * lt:128 * lt + 128],
                                        w1_sb[:, 0, sw, 0:2, 0:512],
                                        start=(ti == 0), stop=False,
                                        perf_mode=DR,
                                    )
                            continue
                        tensor.wait_ge(sems[f"xt_d0{k2}b{buf}"], nth)
                        tensor.wait_ge(sems[f"xt_d1{k2}b{buf}"], nth)
                        def w1_wait(name, short):
                            if it == 0:
                                tensor.wait_ge(sems[f"w0_{short}s0"], 16)
                                tensor.wait_ge(sems[f"w0_{short}s1"], 16)
                            else:
                                nw = ((it + 1) // 2) * 16
                                tensor.wait_ge(sems[f"{name}s0b{buf}"], nw)
                                tensor.wait_ge(sems[f"{name}s1b{buf}"], nw)
                        if k2 == 0:
                            w1_wait("w1a_d0", "d0")
                        elif k2 == 1:
                            w1_wait("w1a_g1", "g1")
                        elif k2 == 2:
                            w1_wait("w1a_g2", "g2")
                        for lt in range(LT):
                            if k2 == 0 and it > 0:
                                # banks 0-4 <- ph3 m=0,1,4,5,6 copies (it-1)
                                for s, c in cp3_waits([0, 1, 4, 5, 6][lt],
                                                      it - 1):
                                    tensor.wait_ge(s, c)
                            mm = proj1_mms(ps[lt], buf, k2, lt, 0)
                            if k2 == K2T - 1:
                                mm.then_inc(sems["mmA"])
                    # ---- wave B: v[:, 512:1024], k-inner, banks [5,6,7,0,1] ----
                    for lt in range(LT):
                        bank = ps[WAVE_B_BANKS[lt]]
                        if lt < 3:
                            if it > 0:  # banks 5,6,7 <- ph3 m=7,2,3 (it-1)
                                for s, c in cp3_waits([7, 2, 3][lt], it - 1):
                                    tensor.wait_ge(s, c)
                        elif lt == 3:
                            # bank 0 <- wave A lt=0 copy of this iter
                            tensor.wait_ge(sems["cpA"], it * LT + 1)
                        else:
                            # bank 1 <- wave A lt=1 copy of this iter
                            tensor.wait_ge(sems["cpA"], it * LT + 2)
                        for k2 in range(K2T):
                            if lt == 0 and k2 == 0:
                                if it == 0:
                                    tensor.wait_ge(sems["w0_bs0"], 16)
                                    tensor.wait_ge(sems["w0_bs1"], 16)
                                else:
                                    nw = ((it + 1) // 2) * 16
                                    tensor.wait_ge(sems[f"w1bs0b{buf}"], nw)
                                    tensor.wait_ge(sems[f"w1bs1b{buf}"], nw)
                            mm = proj1_mms(bank, buf, k2, lt, 512)
                        mm.then_inc(sems["mm1"])

                    # ---- phase 2: attendedT per head (banded, bf16), banks 2-5 ----
                    if it == 0:
                        tensor.wait_ge(sems["tp_d"], 16)

                    def ph2_head(h):
                        bank = ps[PH2_BANKS[h % 4]]
                        # bank WAR: banks 2,3,4 <- wave A lt=2,3,4 copies;
                        # bank 5 <- wave B g=0 copy; h>=4 <- ph2 head h-4
                        # (the at_bf copy is the bank's last reader)
                        if h == 0:
                            tensor.wait_ge(sems["cpA"], it * LT + 3)
                        elif h == 1:
                            tensor.wait_ge(sems["cpA"], it * LT + 4)
                        elif h == 2:
                            tensor.wait_ge(sems["cpA"], it * LT + 5)
                        elif h == 3:
                            tensor.wait_ge(sems["cpB"], it * 4 + 1)
                        else:
                            s, c = at_bank_wait(h - 4, it)
                            tensor.wait_ge(s, c)
                        windows = attn_windows(h) if banded else [
                            (t, 0, NQ) for t in range(LT)]
                        for wi, (t, j0, j1) in enumerate(windows):
                            # data: v tile t, n-chunk h//4. Heads 1-3/5-7
                            # repeat head 0/4's thresholds -- skip the waits
                            if h == 0:
                                tensor.wait_ge(sems["cpA"], it * LT + t + 1)
                            elif h == 4 and t < 4:
                                tensor.wait_ge(sems["cpB"], it * 4 + t + 1)
                            elif h == 4:
                                tensor.wait_ge(sems["cpB4"], it + 1)
                            c0 = 512 - 128 * t + j0 - TP0
                            c1 = 512 - 128 * t + j1 - TP0
                            mm = tensor.matmul(
                                bank[:, j0:j1],
                                v_sb[:, t, 128 * h:128 * h + 128],
                                tp_sb[:, h, c0:c1],
                                start=(wi == 0), stop=(wi == len(windows) - 1),
                            )
                            if wi == len(windows) - 1:
                                mm.then_inc(sems["mm2"])

                    # ---- phase 3: outT = W2 @ attendedT (fp8) ----
                    # hybrid order: k2-outer for the first two passes (gives
                    # each head pair's readout a full pass of slack), then
                    # m-outer for k2=2,3 so the 8 psum groups close staggered
                    # and the copy/DMA drain pipeline overlaps the matmuls.
                    def ph3_mm(k2, m, ti, sa, sw):
                        at = ah_sb if sa == 0 else al_sb
                        return tensor.matmul(
                            ps[PH3_BANKS[m]][:, :],
                            w2_sb[:, sw, 2 * k2:2 * k2 + 2,
                                  128 * m:128 * m + 128],
                            at[:, 2 * k2:2 * k2 + 2, :],
                            start=(k2 == 0 and ti == 0),
                            stop=(k2 == K2T - 1 and ti == len(TERMS) - 1),
                            perf_mode=DR,
                        )

                    # ph2 heads 0-3 (v chunk 0 only)
                    for h in range(4):
                        ph2_head(h)
                    # ph3 k2=0 for m0-3: T1/T3 need only at_hi of heads 0,1;
                    # the T2 block (at_lo) follows once the pool sub lands
                    tensor.wait_ge(sems["w2_s0"], (it + 1) * 16)
                    tensor.wait_ge(sems["w2_s1"], (it + 1) * 16)
                    for s, c in (at_hi_wait(0, it), at_hi_wait(1, it)):
                        tensor.wait_ge(s, c)
                    for m in range(4):
                        # bank WAR: banks 0,1,6,7 <- wave B copies lt=3,4,1,2
                        if m == 1:
                            tensor.wait_ge(sems["cpB4"], it + 1)
                        else:
                            tensor.wait_ge(sems["cpB"],
                                           it * 4 + [4, 0, 2, 3][m])
                        for ti in (0, 2):
                            ph3_mm(0, m, ti, *TERMS[ti])
                    # ph3 T1/T3 (at_hi only) for m0-3 at k2=1
                    for s, c in (at_hi_wait(2, it), at_hi_wait(3, it)):
                        tensor.wait_ge(s, c)
                    for m in range(4):
                        for ti in (0, 2):
                            ph3_mm(1, m, ti, *TERMS[ti])
                    # ph2 heads 4-7 (banks freed by heads 0-3 readouts)
                    for h in range(4, G2):
                        ph2_head(h)
                    # deferred T2 (at_lo) terms for m0-3 -- placed here so
                    # the h4-7 readouts have time to land before ph3b
                    for h in (0, 1, 2, 3):
                        s, c = at_lo_wait(h, it)
                        tensor.wait_ge(s, c)
                    for k2 in (0, 1):
                        for m in range(4):
                            ph3_mm(k2, m, 1, *TERMS[1])
                    # ph3 m4-7: full k2=0 triplets, then k2=1 triplets
                    for m in range(4, G3):
                        s, c = at_bank_wait(m, it)
                        tensor.wait_ge(s, c)
                        for ti, (sa, sw) in enumerate(TERMS):
                            ph3_mm(0, m, ti, sa, sw)
                    for m in range(4, G3):
                        for ti, (sa, sw) in enumerate(TERMS):
                            ph3_mm(1, m, ti, sa, sw)
                    # drain: m-outer k2=2,3; groups close staggered so the
                    # copy/DMA pipeline overlaps the matmuls. Heads 6,7 are
                    # first needed by m0's k2=3 terms -- wait lazily.
                    for h in (4, 5):
                        s, c = at_lo_wait(h, it)
                        tensor.wait_ge(s, c)
                    for ti, (sa, sw) in enumerate(TERMS):
                        ph3_mm(2, 0, ti, sa, sw)
                    for h in (6, 7):
                        s, c = at_lo_wait(h, it)
                        tensor.wait_ge(s, c)
                    for ti, (sa, sw) in enumerate(TERMS):
                        mm = ph3_mm(3, 0, ti, sa, sw)
                    mm.then_inc(sems["mm3"])
                    for m in range(1, G3):
                        for k2 in (2, 3):
                            for ti, (sa, sw) in enumerate(TERMS):
                                mm = ph3_mm(k2, m, ti, sa, sw)
                        mm.then_inc(sems["mm3"])

            @block.vector
            def _(vector: bass.BassEngine):
                for it in range(iters):
                    # wave A copies: v[:, lt, 0:512], banks 0-4, after last MM
                    for lt in range(LT):
                        vector.wait_ge(sems["mmA"], it * LT + lt + 1)
                        vector.tensor_copy(
                            out=v_sb[:, lt, 0:512], in_=ps[lt][:, :],
                        ).then_inc(sems["cpA"])
                    # wave B lt=4 copy (the pool cannot read psum)
                    vector.wait_ge(sems["mm1"], it * LT + 5)
                    vector.tensor_copy(
                        out=v_sb[:, 4, 512:1024],
                        in_=ps[WAVE_B_BANKS[4]][:, :],
                    ).then_inc(sems["cpB4"])
                    # at_hi / at_bf for odd heads (ah h1/h3 front-loaded)
                    def v_ah(h):
                        vector.tensor_copy(
                            out=ah_sb[:, h, :], in_=ps[PH2_BANKS[h % 4]][:, :],
                        ).then_inc(sems["c2hv"])

                    def v_ab(h):
                        vector.tensor_copy(
                            out=ab_sb[:, h, :], in_=ps[PH2_BANKS[h % 4]][:, :],
                        ).then_inc(sems["c2av"])

                    def v_ah_sbuf(h):
                        vector.tensor_copy(
                            out=ah_sb[:, h, :], in_=ab_sb[:, h, :],
                        ).then_inc(sems["c2hv"])

                    vector.wait_ge(sems["mm2"], it * G2 + 2)
                    v_ah(1)
                    vector.wait_ge(sems["mm2"], it * G2 + 4)
                    v_ah(3)
                    v_ab(1)
                    v_ab(3)
                    vector.wait_ge(sems["mm2"], it * G2 + 6)
                    v_ab(5)
                    v_ah_sbuf(5)
                    vector.wait_ge(sems["mm2"], it * G2 + 8)
                    v_ab(7)
                    v_ah_sbuf(7)
                    for m in [0, 2, 4, 6]:
                        vector.wait_ge(sems["mm3"], it * G3 + m + 1)
                        if it > 0:
                            vector.wait_ge(sems[f"dmo{m}"], it * 16)
                        vector.tensor_copy(
                            out=o_sb[:, m, :],
                            in_=ps[PH3_BANKS[m]][:, :],
                        ).then_inc(sems["cp3v"])


            @block.gpsimd
            def _(gpsimd: bass.BassEngine):
                # zero the PE-warmup tile before anything else (fast on pool)
                gpsimd.memset(zdum[:], 0).then_inc(sems["zd"])
                w1_dmas(gpsimd, 0)
                w2_dmas(gpsimd, 0)
                for it in range(iters):
                    # at_lo = fp8(at_bf - at_hi), SBUF-only (pool cannot
                    # access psum on hardware). For h>=4 at_hi is derived
                    # from at_bf and lands second -- wait on it too.
                    for h in range(G2):
                        s, c = at_bank_wait(h, it)
                        gpsimd.wait_ge(s, c)
                        if h >= 4:
                            s, c = at_hi_wait(h, it)
                            gpsimd.wait_ge(s, c)
                        gpsimd.tensor_sub(
                            al_sb[:, h, :], ab_sb[:, h, :], ah_sb[:, h, :],
                        ).then_inc(sems["c2lp"])
                    # ---- output DMAs (m7 goes out on the scalar HWDGE) ----
                    for m in range(G3 - 1):
                        for s, c in cp3_waits(m, it):
                            gpsimd.wait_ge(s, c)
                        gpsimd.dma_start(
                            out=out[128 * m:128 * m + 128, :],
                            in_=o_sb[:, m, :],
                        ).then_inc(sems[f"dmo{m}"], 16)
                    # w2 for the next iter: deadline is its ph3, so after
                    # the outs is fine (mm3 guard is already satisfied here)
                    if it + 1 < iters:
                        w2_dmas(gpsimd, it + 1)
                for m in range(G3 - 1):
                    gpsimd.wait_ge(sems[f"dmo{m}"], iters * 16)
                gpsimd.wait_ge(sems["dmo7s"], iters * 16)

            @block.scalar
            def _(scalar: bass.BassEngine):
                # iter 0's xt lo-half k-pair 0: separate HWDGE queue so it
                # is not behind the hi-half chunks on the sync queue
                scalar.dma_start(
                    out=xt_sb[:, 0, 1, 0:2, :], in_=xt_r[:, 1, 0:2, :],
                ).then_inc(sems["xt_d10b0"], 16)
                for it in range(iters):
                    # wave B copies lt 0-3 (lt 4 runs on the pool so the
                    # at_hi chain isn't stuck behind it)
                    for lt in range(4):
                        scalar.wait_ge(sems["mm1"], it * LT + lt + 1)
                        scalar.copy(v_sb[:, lt, 512:1024],
                                    ps[WAVE_B_BANKS[lt]][:, :]).then_inc(sems["cpB"])
                    # at_hi (fp8) / at_bf (bf16) for even heads; the pool
                    # derives at_lo = at_bf - at_hi off the psum path.
                    # ah h0/h2 are front-loaded: they gate ph3's T1/T3.
                    def s_ah(h):
                        scalar.copy(ah_sb[:, h, :],
                                    ps[PH2_BANKS[h % 4]][:, :]).then_inc(
                            sems["c2hs"])

                    def s_ab(h):
                        scalar.copy(ab_sb[:, h, :],
                                    ps[PH2_BANKS[h % 4]][:, :]).then_inc(
                            sems["c2as"])

                    def s_ah_sbuf(h):
                        # late heads: derive at_hi from the bf16 copy so the
                        # psum bank is freed by the single at_bf read
                        scalar.copy(ah_sb[:, h, :],
                                    ab_sb[:, h, :]).then_inc(sems["c2hs"])

                    scalar.wait_ge(sems["mm2"], it * G2 + 1)
                    s_ah(0)
                    scalar.wait_ge(sems["mm2"], it * G2 + 3)
                    s_ah(2)
                    s_ab(0)
                    s_ab(2)
                    scalar.wait_ge(sems["mm2"], it * G2 + 5)
                    s_ab(4)
                    s_ah_sbuf(4)
                    scalar.wait_ge(sems["mm2"], it * G2 + 7)
                    s_ab(6)
                    s_ah_sbuf(6)
                    for m in [1, 3, 5, 7]:
                        scalar.wait_ge(sems["mm3"], it * G3 + m + 1)
                        if it > 0:
                            scalar.wait_ge(
                                sems["dmo7s" if m == 7 else f"dmo{m}"],
                                it * 16)
                        scalar.copy(o_sb[:, m, :],
                                    ps[PH3_BANKS[m]][:, :]).then_inc(
                            sems["cp3s"])
                        if m == 7:
                            # m7 is the tail-critical group: its output DMA
                            # goes out on this (otherwise idle) HWDGE queue
                            # instead of the pool's SWDGE, whose end-of-
                            # kernel drain would add ~0.5us to the barrier
                            scalar.dma_start(
                                out=out[896:1024, :],
                                in_=o_sb[:, 7, :],
                            ).then_inc(sems["dmo7s"], 16)


    return nc


# ---------------- host side ----------------

_GRAPH_CACHE: dict = {}


def get_graph(iters: int = 1, banded: bool = True) -> bass.Bass:
    key = (iters, banded)
    if key not in _GRAPH_CACHE:
        _GRAPH_CACHE[key] = build_graph(iters, banded)
    return _GRAPH_CACHE[key]


class Runner:
    """Compile-once executor for one Bass graph across the 8 cores.

    Mirrors bass2jax.run_bass_via_pjrt but keeps the jitted callable so
    repeated invocations don't re-trace/re-compile.
    """

    def __init__(self, nc: bass.Bass, n_cores: int = N_CORES):
        import jax
        from jax.sharding import Mesh, PartitionSpec
        from jax.experimental.shard_map import shard_map
        from concourse import bass2jax, mybir as _mb

        bass2jax.install_neuronx_cc_hook()
        self.n_cores = n_cores

        partition_name = (nc.partition_id_tensor.name
                          if nc.partition_id_tensor else None)
        in_names, out_names, out_avals, zero_shapes = [], [], [], []
        for alloc in nc.m.functions[0].allocations:
            if not isinstance(alloc, _mb.MemoryLocationSet):
                continue
            name = alloc.memorylocations[0].name
            if alloc.kind == "ExternalInput":
                if name != partition_name:
                    in_names.append(name)
            elif alloc.kind == "ExternalOutput":
                out_names.append(name)
                shape = tuple(alloc.tensor_shape)
                dtype = _mb.dt.np(alloc.dtype)
                out_avals.append(jax.core.ShapedArray(shape, dtype))
                zero_shapes.append((shape, dtype))
        self.in_names = list(in_names)
        self.out_names = out_names
        self.out_avals = out_avals
        self.zero_shapes = zero_shapes
        n_params = len(in_names)
        all_names = in_names + out_names
        if partition_name is not None:
            all_names = all_names + [partition_name]

        def _body(*args):
            operands = list(args)
            if partition_name is not None:
                operands.append(bass2jax.partition_id_tensor())
            outs = bass2jax._bass_exec_p.bind(
                *operands,
                out_avals=tuple(out_avals),
                in_names=tuple(all_names),
                out_names=tuple(out_names),
                lowering_input_output_aliases=(),
                sim_require_finite=True,
                sim_require_nnan=True,
                nc=nc,
            )
            return tuple(outs)

        devices = jax.devices()[:n_cores]
        mesh = Mesh(np.asarray(devices), ("core",))
        self._mesh = mesh
        n_outs = len(out_names)
        self._fn = jax.jit(
            shard_map(_body, mesh=mesh,
                      in_specs=(PartitionSpec("core"),) * (n_params + n_outs),
                      out_specs=(PartitionSpec("core"),) * n_outs,
                      check_rep=False),
            donate_argnums=tuple(range(n_params, n_params + n_outs)),
            keep_unused=True,
        )

    def stage(self, in_maps):
        """device_put the concatenated inputs once; returns device arrays."""
        import jax
        concat_in = [
            np.concatenate([np.asarray(m[name]) for m in in_maps], axis=0)
            for name in self.in_names
        ]
        return [jax.device_put(a) for a in concat_in]

    def make_zeros(self):
        if not hasattr(self, "_zeros_fn"):
            import jax
            import jax.numpy as jnp
            from jax.sharding import NamedSharding, PartitionSpec
            shardings = tuple(
                NamedSharding(self._mesh, PartitionSpec("core"))
                for _ in self.zero_shapes)
            shapes = [((self.n_cores * s[0], *s[1:]), d)
                      for s, d in self.zero_shapes]

            def _mk():
                return tuple(jnp.zeros(sh, dt) for sh, dt in shapes)

            self._zeros_fn = jax.jit(_mk, out_shardings=shardings)
        return list(self._zeros_fn())

    def run_staged(self, dev_in, dev_zeros):
        return self._fn(*dev_in, *dev_zeros)

    def __call__(self, in_maps):
        out_arrs = self._fn(*self.stage(in_maps), *self.make_zeros())
        return [
            {name: np.asarray(out_arrs[i]).reshape(
                self.n_cores, *self.out_avals[i].shape)[c]
             for i, name in enumerate(self.out_names)}
            for c in range(self.n_cores)
        ]


_RUNNER_CACHE: dict = {}


def get_runner(iters: int = 1) -> "Runner":
    if iters not in _RUNNER_CACHE:
        _RUNNER_CACHE[iters] = Runner(get_graph(iters))
    return _RUNNER_CACHE[iters]


def _hilo(a: np.ndarray) -> tuple[np.ndarray, np.ndarray]:
    """fp8 hi/lo split: a ~= hi + lo with hi = fp8(a), lo = fp8(a - hi)."""
    hi = a.astype(NP_FP8)
    lo = (a - hi.astype(np.float32)).astype(NP_FP8)
    return hi, lo


def make_in_maps(values: np.ndarray, input_weights: np.ndarray,
                 output_weight: np.ndarray) -> list:
    w1h, w1l = _hilo(WSCALE * input_weights.T.astype(np.float32))
    w1t = np.concatenate([w1h, w1l], axis=0)          # [2048, 1024] fp8
    w2h, w2l = _hilo(WSCALE * output_weight.T.astype(np.float32))
    w2t = np.concatenate([w2h, w2l], axis=0)
    tpt = gauss_toeplitz_table()
    in_maps = []
    for core in range(N_CORES):
        b, c = divmod(core, 4)
        lo, hi = c * CHUNK - HALO_L, c * CHUNK + CHUNK + HALO_R
        src_lo, src_hi = max(lo, 0), min(hi, L)
        xt_pad = np.zeros((2 * D, LPAD), dtype=NP_FP8)
        xs = values[b, src_lo:src_hi, :].T.astype(np.float32)
        xh, xl = _hilo(xs)
        xt_pad[0:D, src_lo - lo:src_hi - lo] = xh
        xt_pad[D:2 * D, src_lo - lo:src_hi - lo] = xl
        in_maps.append({"xt": xt_pad, "w1t": w1t, "w2t": w2t, "tp": tpt})
    return in_maps


def assemble(results: list) -> np.ndarray:
    out = np.empty((B, L, D), dtype=np.float32)
    inv = np.float32(1.0 / WSCALE)
    for core in range(N_CORES):
        b, c = divmod(core, 4)
        out[b, c * CHUNK:(c + 1) * CHUNK, :] = \
            results[core]["out"].T.astype(np.float32) * inv
    return out


def kernel(values: np.ndarray, input_weights: np.ndarray,
           output_weight: np.ndarray) -> np.ndarray:
    in_maps = make_in_maps(values, input_weights, output_weight)
    try:
        return assemble(get_runner(1)(in_maps))
    except Exception:
        # fallback: canonical SPMD path (re-traces per call but always works)
        res = run_bass_kernel_spmd(get_graph(1), in_maps,
                                   core_ids=list(range(N_CORES)))
        return assemble(res.results)
